# revision 1
# baseline (speedup 1.0000x reference)
"""MLA (multi-head latent attention) Bass kernel for Trainium2, 8 NeuronCores.

Sharding: core i handles batch b = i // 2 and head-group g = i % 2
(8 of the 16 heads).  Each core computes a partial output
(its heads' contribution through out_proj, plus b_o/2); the host sums
the two partials per batch.

Layout strategy (all on-chip tensors "t-major", i.e. feature dim on
partitions, sequence on the free axis):
  xT      [dim=8x128, S]   via PE (tensor-engine) transposes of x
  kv_latT [128, S]         = w_kvc^T @ xT        (+b_kvc)
  q_latT  [256, S]         = w_qc^T @ xT         (+b_qc)
  KT      [512, S]         = w_kvu_k^T @ kv_latT (+b)    (local heads)
  QT      [512, S]         = w_qu^T   @ q_latT   (+b)
  V       [S, 520]         = kv_lat @ w_kvu_v    (+b), 65-col blocks per
                             head: 64 value cols + a ones column.
Attention per (s-half j, head pair), streaming over key chunks k:
  scoresT[t,s] via matmul (head pair shares the PE array via disjoint
  64-row groups), exp(s/8) on ScalarE, causal handled by clipping the
  s-range + affine_select on the diagonal block; PV accumulates
  ctx^T[64, s] in PSUM, the ones column gives the softmax denominator
  in row 64.  ctx scaled by 1/denom (reciprocal + partition-broadcast
  multiply) into ctxT, then out = ctxT^T @ w_o + b_o/2.

Matmul operands use float32r (single-pass fp32 streaming on the PE,
4x faster than exact fp32); producers write tiles with f32r dtype so
operands are pre-rounded.
"""

import numpy as np

import concourse.bass as bass
import concourse.bacc as bacc
import concourse.mybir as mybir
import concourse.tile as tile
from concourse import masks

DIM = 1024
NUM_HEADS = 16
HEAD_DIM = 64
LAT = 128
QR = 256
B = 4
NCORES = 8
ND = DIM // 128       # 8 d-chunks
NHL = 8               # heads per core
F32 = mybir.dt.float32
F32R = mybir.dt.float32r
AF = mybir.ActivationFunctionType


def _pieces(total, w=512):
    return [(o, min(w, total - o)) for o in range(0, total, w)]


def build_mla(S=2048, mmdt=F32R):
    """Build the per-core Bass program (same SPMD program on all 8 cores)."""
    assert S % 256 == 0
    SH = S // 2           # s-half width
    NT = S // 128         # number of 128-token chunks

    nc = bacc.Bacc()

    x_d = nc.declare_dram_parameter("x", [S, DIM], F32, isOutput=False)
    w_kvc_d = nc.declare_dram_parameter("w_kvc", [DIM, LAT], F32, isOutput=False)
    w_qc_d = nc.declare_dram_parameter("w_qc", [DIM, QR], F32, isOutput=False)
    w_kvu_k_d = nc.declare_dram_parameter("w_kvu_k", [LAT, 512], F32, isOutput=False)
    w_kvu_v_d = nc.declare_dram_parameter("w_kvu_v", [LAT, 512], F32, isOutput=False)
    w_qu_d = nc.declare_dram_parameter("w_qu", [QR, 512], F32, isOutput=False)
    w_o_d = nc.declare_dram_parameter("w_o", [512, DIM], F32, isOutput=False)
    b_kvc_d = nc.declare_dram_parameter("b_kvc", [LAT, 1], F32, isOutput=False)
    b_qc_d = nc.declare_dram_parameter("b_qc", [128, 2], F32, isOutput=False)
    b_qu_d = nc.declare_dram_parameter("b_qu", [128, 4], F32, isOutput=False)
    b_kvu_k_d = nc.declare_dram_parameter("b_kvu_k", [128, 4], F32, isOutput=False)
    b_kvu_v_d = nc.declare_dram_parameter("b_kvu_v", [1, 512], F32, isOutput=False)
    b_o_d = nc.declare_dram_parameter("b_o", [1, DIM], F32, isOutput=False)
    out_d = nc.declare_dram_parameter("out", [S, DIM], F32, isOutput=True)

    with tile.TileContext(nc) as tc:
        with (
            tc.tile_pool(name="const", bufs=1) as const,
            tc.tile_pool(name="wts", bufs=1) as wts,
            tc.tile_pool(name="big", bufs=1) as big,
            tc.tile_pool(name="stg", bufs=2) as stg,
        ):
            ident = const.tile([128, 128], F32, name="ident")
            masks.make_identity(nc, ident[:])
            # memset doesn't support f32r; memset f32 then round-copy
            ones1f = const.tile([1, 128], F32, name="ones1f")
            nc.gpsimd.memset(ones1f[:], 1.0)
            ones1 = const.tile([1, 128], mmdt, name="ones1")
            nc.vector.tensor_copy(ones1[:], ones1f[:])

            # ---- weights into SBUF (staged fp32 DMA, rounded copy to mmdt) --
            def load_rounded(dst_ap, src_ap, shape):
                st = stg.tile([128, 1024], F32, tag="stage")
                sap = st[:shape[0], :shape[1]]
                nc.sync.dma_start(out=sap, in_=src_ap)
                nc.vector.tensor_copy(dst_ap, sap)

            w_kvc_sb = wts.tile([128, DIM], mmdt, name="w_kvc_sb")
            w_qc_sb = wts.tile([128, ND * QR], mmdt, name="w_qc_sb")
            for dc in range(ND):
                load_rounded(w_kvc_sb[:, 128 * dc:128 * dc + 128],
                             w_kvc_d[128 * dc:128 * dc + 128, :], (128, 128))
                load_rounded(w_qc_sb[:, QR * dc:QR * dc + QR],
                             w_qc_d[128 * dc:128 * dc + 128, :], (128, QR))
            w_kvu_k_sb = wts.tile([128, 512], mmdt, name="w_kvu_k_sb")
            load_rounded(w_kvu_k_sb[:], w_kvu_k_d[:, :], (128, 512))
            w_kvu_v_sb = wts.tile([128, 512], mmdt, name="w_kvu_v_sb")
            load_rounded(w_kvu_v_sb[:], w_kvu_v_d[:, :], (128, 512))
            w_qu_sb = wts.tile([128, 1024], mmdt, name="w_qu_sb")
            for qc in range(2):
                load_rounded(w_qu_sb[:, 512 * qc:512 * qc + 512],
                             w_qu_d[128 * qc:128 * qc + 128, :], (128, 512))
            b_kvu_v_sb = wts.tile([1, 512], mmdt, name="b_kvu_v_sb")
            load_rounded(b_kvu_v_sb[:], b_kvu_v_d[:, :], (1, 512))
            b_o_sb = wts.tile([1, DIM], mmdt, name="b_o_sb")
            load_rounded(b_o_sb[:], b_o_d[:, :], (1, DIM))
            # preload w_o so phase E starts without waiting on its DMA
            w_o_sb = wts.tile([128, 4 * DIM], mmdt, name="w_o_sb")
            for cc in range(4):
                load_rounded(w_o_sb[:, DIM * cc:DIM * cc + DIM],
                             w_o_d[128 * cc:128 * cc + 128, :], (128, DIM))

            # per-partition bias vectors (not matmul operands -> plain f32)
            b_kvc_sb = wts.tile([128, 1], F32, name="b_kvc_sb")
            nc.sync.dma_start(out=b_kvc_sb[:], in_=b_kvc_d[:, :])
            b_qc_sb = wts.tile([128, 2], F32, name="b_qc_sb")
            nc.sync.dma_start(out=b_qc_sb[:], in_=b_qc_d[:, :])
            b_qu_sb = wts.tile([128, 4], F32, name="b_qu_sb")
            nc.sync.dma_start(out=b_qu_sb[:], in_=b_qu_d[:, :])
            b_kvu_k_sb = wts.tile([128, 4], F32, name="b_kvu_k_sb")
            nc.sync.dma_start(out=b_kvu_k_sb[:], in_=b_kvu_k_d[:, :])

            # ---- persistent products: KT / QT / V (chunk c lives at cols c*S) ----
            KT = big.tile([128, 4 * S], mmdt, name="KT")
            QT = big.tile([128, 4 * S], mmdt, name="QT")
            V = big.tile([128, NT * 520], mmdt, name="V")
            # ones columns of V (col 64 of each 65-wide head block);
            # memset doesn't support f32r, so copy from an f32 ones tile
            v_view = V[:].rearrange("p (k h c) -> p k h c", h=NHL, c=65)
            ones_cols = const.tile([128, NT * NHL], F32, name="ones_cols")
            nc.gpsimd.memset(ones_cols[:], 1.0)
            nc.vector.tensor_copy(
                v_view[:, :, :, 64:65],
                ones_cols[:].rearrange("p (k h o) -> p k h o", h=NHL, o=1))

            # ================= phase A+B+C: transpose + projections =========
            with (
                tc.tile_pool(name="xin", bufs=3) as xin,
                tc.tile_pool(name="xtp", bufs=2) as xtp,
                tc.tile_pool(name="kvq", bufs=2) as kvq,
                tc.tile_pool(name="tpps", bufs=1, space="PSUM") as tpps,
                tc.tile_pool(name="pjps", bufs=1, space="PSUM") as pjps,
            ):
                for off, w in _pieces(S):
                    ntile = w // 128
                    # transpose x rows [off, off+w) -> xTp [128, 8 * w]
                    # (d-chunk dc at cols dc*w)
                    xTp = xtp.tile([128, ND * 512], mmdt, tag="xTp")
                    for q in range(ntile):
                        xt = xin.tile([128, DIM], F32, tag="xin")
                        nc.sync.dma_start(
                            out=xt[:],
                            in_=x_d[off + 128 * q:off + 128 * q + 128, :])
                        for dg in range(2):
                            ps = tpps.tile([128, 512], F32, tag="tp", bufs=2)
                            for u in range(4):
                                dc = 4 * dg + u
                                nc.tensor.transpose(
                                    ps[:, 128 * u:128 * u + 128],
                                    xt[:, 128 * dc:128 * dc + 128],
                                    ident[:])
                            dst = xTp[:].rearrange(
                                "p (d t) -> p d t", t=512
                            )[:, 4 * dg:4 * dg + 4, 128 * q:128 * q + 128]
                            src = ps[:].rearrange("p (d t) -> p d t", t=128)
                            nc.vector.tensor_copy(dst, src)
                    # kv_lat / q_lat for this piece
                    kvp = pjps.tile([128, 512], F32, tag="kv", bufs=1)
                    q0p = pjps.tile([128, 512], F32, tag="q0", bufs=1)
                    q1p = pjps.tile([128, 512], F32, tag="q1", bufs=1)
                    for dc in range(ND):
                        xr = xTp[:, dc * 512:dc * 512 + w]
                        st = dc == 0
                        sp = dc == ND - 1
                        nc.tensor.matmul(
                            kvp[:, :w], w_kvc_sb[:, 128 * dc:128 * dc + 128],
                            xr, start=st, stop=sp)
                        nc.tensor.matmul(
                            q0p[:, :w], w_qc_sb[:, QR * dc:QR * dc + 128],
                            xr, start=st, stop=sp)
                        nc.tensor.matmul(
                            q1p[:, :w], w_qc_sb[:, QR * dc + 128:QR * dc + 256],
                            xr, start=st, stop=sp)
                    kvs = kvq.tile([128, 512], mmdt, tag="kvs")
                    q0s = kvq.tile([128, 512], mmdt, tag="q0s")
                    q1s = kvq.tile([128, 512], mmdt, tag="q1s")
                    nc.vector.tensor_scalar_add(kvs[:, :w], kvp[:, :w], b_kvc_sb[:, 0:1])
                    nc.vector.tensor_scalar_add(q0s[:, :w], q0p[:, :w], b_qc_sb[:, 0:1])
                    nc.vector.tensor_scalar_add(q1s[:, :w], q1p[:, :w], b_qc_sb[:, 1:2])
                    # K^T / Q^T chunks for this piece
                    for c in range(4):
                        kp = pjps.tile([128, 512], F32, tag="pjo", bufs=2)
                        nc.tensor.matmul(
                            kp[:, :w], w_kvu_k_sb[:, 128 * c:128 * c + 128],
                            kvs[:, :w], start=True, stop=True)
                        nc.vector.tensor_scalar_add(
                            KT[:, c * S + off:c * S + off + w], kp[:, :w],
                            b_kvu_k_sb[:, c:c + 1])
                        qp = pjps.tile([128, 512], F32, tag="pjo", bufs=2)
                        nc.tensor.matmul(
                            qp[:, :w], w_qu_sb[:, 128 * c:128 * c + 128],
                            q0s[:, :w], start=True, stop=False)
                        nc.tensor.matmul(
                            qp[:, :w], w_qu_sb[:, 512 + 128 * c:512 + 128 * c + 128],
                            q1s[:, :w], start=False, stop=True)
                        nc.vector.tensor_scalar_add(
                            QT[:, c * S + off:c * S + off + w], qp[:, :w],
                            b_qu_sb[:, c:c + 1])
                    # V chunks for this piece
                    for q in range(ntile):
                        k = (off + 128 * q) // 128
                        vp = pjps.tile([128, 512], F32, tag="pjo", bufs=2)
                        nc.tensor.matmul(vp[:], ones1[0:1, :], b_kvu_v_sb[0:1, :],
                                         start=True, stop=False)
                        nc.tensor.matmul(vp[:], kvs[:, 128 * q:128 * q + 128],
                                         w_kvu_v_sb[:], start=False, stop=True)
                        nc.vector.tensor_copy(
                            v_view[:, k, :, 0:64],
                            vp[:].rearrange("p (h c) -> p h c", c=64))

            # ================= phase D: attention ===========================
            with tc.tile_pool(name="ctxTp", bufs=1) as ctxTp:
                ctxT = ctxTp.tile([128, 4 * S], mmdt, name="ctxT")
                with (
                    tc.tile_pool(name="attn", bufs=1) as attn,
                    tc.tile_pool(name="scps", bufs=1, space="PSUM") as scps,
                    tc.tile_pool(name="ctxps", bufs=2, space="PSUM") as ctxps,
                ):
                    nbank = (SH + 511) // 512
                    for j in range(2):
                        s0 = SH * j
                        kmax = (SH // 128) * (j + 1)
                        last_k = {
                            bi: min(kmax - 1, (s0 + 512 * (bi + 1)) // 128 - 1)
                            for bi in range(nbank)
                        }
                        for hp in range(NHL // 2):
                            heads = (2 * hp, 2 * hp + 1)
                            c = hp // 1  # KT/QT chunk = hp
                            ctxs = [ctxps.tile([65, SH], F32, tag="ctx",
                                               name=f"ctx{h}") for h in heads]
                            for k in range(kmax):
                                t0 = 128 * k
                                ss = max(s0, t0)
                                fd = s0 + SH - ss
                                rel = ss - s0
                                scs = []
                                # the two heads' QK matmuls are adjacent and
                                # use disjoint 64-row groups of the PE array
                                for o2, w2 in _pieces(fd):
                                    for hi, h in enumerate(heads):
                                        po = 64 * (h % 2)
                                        if o2 == 0:
                                            scs.append(scps.tile(
                                                [128, SH], F32, tag="sc",
                                                bufs=2, name=f"sc{h}"))
                                        nc.tensor.matmul(
                                            scs[hi][:, o2:o2 + w2],
                                            KT[po:po + 64,
                                               hp * S + t0:hp * S + t0 + 128],
                                            QT[po:po + 64,
                                               hp * S + ss + o2:hp * S + ss + o2 + w2],
                                            start=True, stop=True)
                                exs = []
                                for hi, h in enumerate(heads):
                                    ex = attn.tile([128, SH], mmdt, tag="ex",
                                                   bufs=4, name=f"ex{h}")
                                    exs.append(ex)
                                    nc.scalar.activation(ex[:, :fd], scs[hi][:, :fd],
                                                         AF.Exp, scale=0.125)
                                    if t0 >= s0:
                                        nc.gpsimd.affine_select(
                                            out=ex[:, 0:128], in_=ex[:, 0:128],
                                            pattern=[[1, 128]],
                                            compare_op=mybir.AluOpType.is_ge,
                                            fill=0.0, base=0, channel_multiplier=-1)
                                for hi, h in enumerate(heads):
                                    for bi in range(nbank):
                                        a = max(rel, 512 * bi)
                                        b2 = min(SH, 512 * bi + 512)
                                        if a >= b2:
                                            continue
                                        nc.tensor.matmul(
                                            ctxs[hi][:, a:b2],
                                            V[:, 520 * k + 65 * h:520 * k + 65 * h + 65],
                                            exs[hi][:, a - rel:b2 - rel],
                                            start=(k == 0), stop=(k == last_k[bi]))
                            # normalize: ctx[0:64] * (1/ctx[64])
                            for hi, h in enumerate(heads):
                                po = 64 * (h % 2)
                                rec = attn.tile([1, SH], F32, tag="rec", bufs=1,
                                                name=f"rec{h}")
                                nc.vector.reciprocal(rec[:], ctxs[hi][64:65, :])
                                rbc = attn.tile([64, SH], F32, tag="rbc", bufs=1,
                                                name=f"rbc{h}")
                                nc.gpsimd.partition_broadcast(rbc[:], rec[0:1, :])
                                nc.vector.tensor_mul(
                                    ctxT[po:po + 64, hp * S + s0:hp * S + s0 + SH],
                                    ctxs[hi][0:64, :], rbc[:])

            # ================= phase E: out projection ======================
                with (
                    tc.tile_pool(name="outsb", bufs=3) as outsb,
                    tc.tile_pool(name="ops", bufs=2, space="PSUM") as ops,
                ):
                    for si in range(NT):
                        op = ops.tile([128, DIM], F32, tag="op")
                        for o2, w2 in _pieces(DIM):
                            nc.tensor.matmul(op[:, o2:o2 + w2], ones1[0:1, :],
                                             b_o_sb[0:1, o2:o2 + w2],
                                             start=True, stop=False)
                        for cc in range(4):
                            for o2, w2 in _pieces(DIM):
                                nc.tensor.matmul(
                                    op[:, o2:o2 + w2],
                                    ctxT[:, cc * S + 128 * si:cc * S + 128 * si + 128],
                                    w_o_sb[:, DIM * cc + o2:DIM * cc + o2 + w2],
                                    start=False, stop=(cc == 3))
                        ob = outsb.tile([128, DIM], F32, tag="ob")
                        nc.vector.tensor_copy(ob[:, 0:512], op[:, 0:512])
                        nc.scalar.copy(ob[:, 512:DIM], op[:, 512:DIM])
                        nc.sync.dma_start(
                            out=out_d[128 * si:128 * si + 128, :], in_=ob[:])

    nc.finalize()
    return nc


def shard_inputs(inputs, S=2048):
    """Build the 8 per-core input maps from full inputs."""
    f = lambda a: np.ascontiguousarray(np.asarray(a, dtype=np.float32))
    x = f(inputs["x"])
    w_kvc, b_kvc = f(inputs["w_kvc"]), f(inputs["b_kvc"])
    w_kvu, b_kvu = f(inputs["w_kvu"]), f(inputs["b_kvu"])
    w_qc, b_qc = f(inputs["w_qc"]), f(inputs["b_qc"])
    w_qu, b_qu = f(inputs["w_qu"]), f(inputs["b_qu"])
    w_o, b_o = f(inputs["w_o"]), f(inputs["b_o"])
    in_maps = []
    for core in range(NCORES):
        b = core // 2
        g = core % 2
        cs = slice(512 * g, 512 * g + 512)
        in_maps.append({
            "x": x[b],
            "w_kvc": w_kvc,
            "w_qc": w_qc,
            "w_kvu_k": np.ascontiguousarray(w_kvu[:, 512 * g:512 * g + 512]),
            "w_kvu_v": np.ascontiguousarray(w_kvu[:, 1024 + 512 * g:1024 + 512 * g + 512]),
            "w_qu": np.ascontiguousarray(w_qu[:, cs]),
            "w_o": np.ascontiguousarray(w_o[cs, :]),
            "b_kvc": b_kvc.reshape(LAT, 1),
            "b_qc": np.ascontiguousarray(b_qc.reshape(2, 128).T),
            "b_qu": np.ascontiguousarray(b_qu[cs].reshape(4, 128).T),
            "b_kvu_k": np.ascontiguousarray(b_kvu[cs].reshape(4, 128).T),
            "b_kvu_v": np.ascontiguousarray(b_kvu[1024 + 512 * g:1024 + 512 * g + 512].reshape(1, 512)),
            "b_o": np.ascontiguousarray((b_o * 0.5).reshape(1, DIM)),
        })
    return in_maps


def kernel(**inputs) -> np.ndarray:
    from concourse.bass_utils import run_bass_kernel_spmd

    x = np.asarray(inputs["x"])
    S = x.shape[1]
    nc = build_mla(S=S)
    in_maps = shard_inputs(inputs, S=S)
    res = run_bass_kernel_spmd(nc, in_maps, list(range(NCORES))).results
    out = np.empty((B, S, DIM), dtype=np.float32)
    for b in range(B):
        out[b] = res[2 * b]["out"] + res[2 * b + 1]["out"]
    return out



# revision 4
# speedup vs baseline: 1.2388x; 1.2388x over previous
"""MLA (multi-head latent attention) Bass kernel for Trainium2, 8 NeuronCores.

Sharding: core i handles batch b = i // 2 and head-group g2 = i % 2
(8 of the 16 heads).  Each core computes a partial output
(its heads' contribution through out_proj, plus b_o/2); the host sums
the two partials per batch.

v2 design (ACT-bound):
  - All heavy host-transformable data arrives pre-laid-out: x transposed
    to xT bf16 [8,128,2048], weights pre-cast bf16, K/Q up-projection
    columns pre-permuted so the fp8 DoubleRow layout falls out of plain
    PSUM evacuations.
  - QK^T runs in fp8e4 DoubleRow perf mode: KT8/QT8 are stored
    [128p, g, plane, S] where partition 32a+p / plane pl encodes head
    4g+a, dim 32*pl+p; one matmul contracts all 64 head dims (2 k-tiles
    of 32) at 0.5 cycles/col.
  - All other matmuls are bf16 (1 cycle/col, no small-output penalty).
  - Softmax exp on ScalarE is the per-core floor (~135us); the emission
    order software-pipelines projections (2nd half) and out-proj tiles
    into the attention loops so PE fills its slack while ACT grinds.
  - PSUM: scores [128,1024]x2bufs (4 banks) + ctx [65,1024] (2 banks)
    + shared [128,512]x2 work tiles for projections/out-proj (2 banks).
  - Bias adds ride the PSUM->SBUF evacuations (DVE tensor_scalar_add,
    Pool tensor_tensor) - no PE bias matmuls.
"""

import numpy as np
import ml_dtypes

import concourse.bass as bass
import concourse.bacc as bacc
import concourse.mybir as mybir
import concourse.tile as tile

DIM = 1024
NUM_HEADS = 16
HEAD_DIM = 64
LAT = 128
QR = 256
B = 4
NCORES = 8
ND = DIM // 128       # 8 d-chunks
NHL = 8               # heads per core
F32 = mybir.dt.float32
BF16 = mybir.dt.bfloat16
FP8 = mybir.dt.float8e4
AF = mybir.ActivationFunctionType
ALU = mybir.AluOpType
DR = mybir.MatmulPerfMode.DoubleRow


def _pieces(total, w=512):
    return [(o, min(w, total - o)) for o in range(0, total, w)]


def build_mla(S=2048):
    """Build the per-core Bass program (same SPMD program on all 8 cores)."""
    assert S % 1024 == 0
    SH = S // 2           # s-half width
    NT = S // 128         # number of 128-token chunks
    NP = S // 512         # number of 512-token pieces

    nc = bacc.Bacc()

    x_d = nc.declare_dram_parameter("x", [ND, 128, S], BF16, isOutput=False)
    w_kvc_d = nc.declare_dram_parameter("w_kvc", [ND, 128, LAT], BF16, isOutput=False)
    w_qc_d = nc.declare_dram_parameter("w_qc", [ND, 128, QR], BF16, isOutput=False)
    w_kvu_k_d = nc.declare_dram_parameter("w_kvu_k", [128, 512], BF16, isOutput=False)
    w_qu_d = nc.declare_dram_parameter("w_qu", [2, 128, 512], BF16, isOutput=False)
    w_kvu_v_d = nc.declare_dram_parameter("w_kvu_v", [128, 512], BF16, isOutput=False)
    w_o_d = nc.declare_dram_parameter("w_o", [4, 128, DIM], BF16, isOutput=False)
    b_kvc_d = nc.declare_dram_parameter("b_kvc", [LAT, 1], F32, isOutput=False)
    b_qc_d = nc.declare_dram_parameter("b_qc", [128, 2], F32, isOutput=False)
    b_qu_d = nc.declare_dram_parameter("b_qu", [128, 4], F32, isOutput=False)
    b_kvu_k_d = nc.declare_dram_parameter("b_kvu_k", [128, 4], F32, isOutput=False)
    b_kvu_v_d = nc.declare_dram_parameter("b_kvu_v", [1, 512], F32, isOutput=False)
    b_o_d = nc.declare_dram_parameter("b_o", [1, DIM], F32, isOutput=False)
    out_d = nc.declare_dram_parameter("out", [S, DIM], F32, isOutput=True)

    with tile.TileContext(nc) as tc:
        with (
            tc.tile_pool(name="wts", bufs=1) as wts,
            tc.tile_pool(name="big", bufs=1) as big,
            tc.tile_pool(name="lat", bufs=2) as latp,
            tc.tile_pool(name="exb", bufs=4) as exb,
            tc.tile_pool(name="nrm", bufs=2) as nrm,
            tc.tile_pool(name="obp", bufs=4) as obp,
            tc.tile_pool(name="psc", bufs=2, space="PSUM") as psc,
            tc.tile_pool(name="pctx", bufs=1, space="PSUM") as pctx,
            tc.tile_pool(name="pwk", bufs=2, space="PSUM") as pwk,
        ):
            # ---- weights straight into SBUF (pre-cast on host) -------------
            w_kvc_sb = wts.tile([128, ND, LAT], BF16, name="w_kvc_sb")
            w_qc_sb = wts.tile([128, ND, QR], BF16, name="w_qc_sb")
            for dc in range(ND):
                nc.sync.dma_start(out=w_kvc_sb[:, dc, :], in_=w_kvc_d[dc, :, :])
                nc.sync.dma_start(out=w_qc_sb[:, dc, :], in_=w_qc_d[dc, :, :])
            w_kvu_k_sb = wts.tile([128, 512], BF16, name="w_kvu_k_sb")
            nc.sync.dma_start(out=w_kvu_k_sb[:], in_=w_kvu_k_d[:, :])
            w_qu_sb = wts.tile([128, 2, 512], BF16, name="w_qu_sb")
            for qh in range(2):
                nc.sync.dma_start(out=w_qu_sb[:, qh, :], in_=w_qu_d[qh, :, :])
            w_kvu_v_sb = wts.tile([128, 512], BF16, name="w_kvu_v_sb")
            nc.sync.dma_start(out=w_kvu_v_sb[:], in_=w_kvu_v_d[:, :])
            w_o_sb = wts.tile([128, 4, DIM], BF16, name="w_o_sb")
            for cc in range(4):
                nc.sync.dma_start(out=w_o_sb[:, cc, :], in_=w_o_d[cc, :, :])

            b_kvc_sb = wts.tile([128, 1], F32, name="b_kvc_sb")
            nc.sync.dma_start(out=b_kvc_sb[:], in_=b_kvc_d[:, :])
            b_qc_sb = wts.tile([128, 2], F32, name="b_qc_sb")
            nc.sync.dma_start(out=b_qc_sb[:], in_=b_qc_d[:, :])
            b_qu_sb = wts.tile([128, 4], F32, name="b_qu_sb")
            nc.sync.dma_start(out=b_qu_sb[:], in_=b_qu_d[:, :])
            b_kvu_k_sb = wts.tile([128, 4], F32, name="b_kvu_k_sb")
            nc.sync.dma_start(out=b_kvu_k_sb[:], in_=b_kvu_k_d[:, :])
            bv_row = wts.tile([1, 512], F32, name="bv_row")
            nc.sync.dma_start(out=bv_row[:], in_=b_kvu_v_d[:, :])
            bvb = wts.tile([128, 512], F32, name="bvb")
            nc.gpsimd.partition_broadcast(bvb[:], bv_row[0:1, :])
            bo_row = wts.tile([1, DIM], F32, name="bo_row")
            nc.sync.dma_start(out=bo_row[:], in_=b_o_d[:, :])
            bob = wts.tile([128, DIM], F32, name="bob")
            nc.gpsimd.partition_broadcast(bob[:], bo_row[0:1, :])

            # ---- persistent tensors ---------------------------------------
            xT = big.tile([128, ND, S], BF16, name="xT")
            for p in range(NP):
                for dc in range(ND):
                    nc.sync.dma_start(
                        out=xT[:, dc, 512 * p:512 * p + 512],
                        in_=x_d[dc, :, 512 * p:512 * p + 512])

            # KT8/QT8: [128p, g, plane, S]; partition 32a+p, plane pl
            # holds head 4g+a, dim 32*pl+p (fp8 for DoubleRow QK).
            KT8 = big.tile([128, 2, 2, S], FP8, name="KT8")
            QT8 = big.tile([128, 2, 2, S], FP8, name="QT8")
            # V: [128tok, chunk, head, 65] (64 vals + ones col)
            V = big.tile([128, NT, NHL, 65], BF16, name="V")
            nc.gpsimd.memset(V[:, :, :, 64:65], 1.0)
            # ctxT: [128 (2 heads x 64 dims), chunk h//2, S]
            ctxT = big.tile([128, 4, S], BF16, name="ctxT")

            # ---- work-unit emitters ---------------------------------------
            def unit_kv(p):
                off = 512 * p
                kvp = pwk.tile([128, 512], F32, tag="wk")
                for dc in range(ND):
                    nc.tensor.matmul(
                        kvp[:], w_kvc_sb[:, dc, :],
                        xT[:, dc, off:off + 512],
                        start=(dc == 0), stop=(dc == ND - 1))
                kvs = latp.tile([128, 512], BF16, tag=f"kvs")
                nc.vector.tensor_scalar_add(kvs[:], kvp[:], b_kvc_sb[:, 0:1])
                return kvs

            def unit_q(p, qh):
                off = 512 * p
                qp = pwk.tile([128, 512], F32, tag="wk")
                for dc in range(ND):
                    nc.tensor.matmul(
                        qp[:], w_qc_sb[:, dc, 128 * qh:128 * qh + 128],
                        xT[:, dc, off:off + 512],
                        start=(dc == 0), stop=(dc == ND - 1))
                qs = latp.tile([128, 512], BF16, tag=f"q{qh}s")
                nc.vector.tensor_scalar_add(qs[:], qp[:], b_qc_sb[:, qh:qh + 1])
                return qs

            def unit_KT(p, j, kvs):
                off = 512 * p
                kp = pwk.tile([128, 512], F32, tag="wk")
                nc.tensor.matmul(kp[:], w_kvu_k_sb[:, 128 * j:128 * j + 128],
                                 kvs[:], start=True, stop=True)
                nc.vector.tensor_scalar_add(
                    KT8[:, j // 2, j % 2, off:off + 512], kp[:],
                    b_kvu_k_sb[:, j:j + 1])

            def unit_QT(p, j, q0s, q1s):
                off = 512 * p
                qp = pwk.tile([128, 512], F32, tag="wk")
                nc.tensor.matmul(qp[:], w_qu_sb[:, 0, 128 * j:128 * j + 128],
                                 q0s[:], start=True, stop=False)
                nc.tensor.matmul(qp[:], w_qu_sb[:, 1, 128 * j:128 * j + 128],
                                 q1s[:], start=False, stop=True)
                nc.vector.tensor_scalar_add(
                    QT8[:, j // 2, j % 2, off:off + 512], qp[:],
                    b_qu_sb[:, j:j + 1])

            def unit_V(p, q, kvs):
                k = 4 * p + q
                vp = pwk.tile([128, 512], F32, tag="wk")
                nc.tensor.matmul(vp[:], kvs[:, 128 * q:128 * q + 128],
                                 w_kvu_v_sb[:], start=True, stop=True)
                nc.vector.tensor_tensor(
                    V[:, k, :, 0:64],
                    vp[:].rearrange("p (h c) -> p h c", c=64),
                    bvb[:].rearrange("p (h c) -> p h c", c=64), ALU.add)

            def piece_units(p):
                state = {}
                yield lambda: state.__setitem__("kvs", unit_kv(p))
                yield lambda: state.__setitem__("q0s", unit_q(p, 0))
                yield lambda: state.__setitem__("q1s", unit_q(p, 1))
                for j in range(4):
                    yield lambda j=j: unit_KT(p, j, state["kvs"])
                for j in range(4):
                    yield lambda j=j: unit_QT(p, j, state["q0s"], state["q1s"])
                for q in range(4):
                    yield lambda q=q: unit_V(p, q, state["kvs"])

            def unit_E(si, o):
                op = pwk.tile([128, 512], F32, tag="wk")
                for cc in range(4):
                    nc.tensor.matmul(
                        op[:], ctxT[:, cc, 128 * si:128 * si + 128],
                        w_o_sb[:, cc, 512 * o:512 * o + 512],
                        start=(cc == 0), stop=(cc == 3))
                ob = obp.tile([128, 512], F32, tag="ob")
                nc.vector.tensor_tensor(ob[:], op[:],
                                        bob[:, 512 * o:512 * o + 512], ALU.add)
                nc.sync.dma_start(
                    out=out_d[128 * si:128 * si + 128, 512 * o:512 * o + 512],
                    in_=ob[:])

            def attn_head(j, h, filler):
                """Attention for s-half j, local head h; pulls filler work
                after each key chunk to keep PE fed while ACT runs exp."""
                s0 = SH * j
                kmax = (SH // 128) * (j + 1)
                nbank = SH // 512
                last_k = {
                    bi: min(kmax - 1, (s0 + 512 * (bi + 1)) // 128 - 1)
                    for bi in range(nbank)
                }
                g, a = h // 4, h % 4
                ctx = pctx.tile([65, SH], F32, tag="ctx")
                for k in range(kmax):
                    t0 = 128 * k
                    ss = max(s0, t0)
                    fd = s0 + SH - ss
                    rel = ss - s0
                    sc = psc.tile([128, SH], F32, tag="sc")
                    for o2, w2 in _pieces(fd, 256):
                        nc.tensor.matmul(
                            sc[:, o2:o2 + w2],
                            KT8[32 * a:32 * a + 32, g, :, t0:t0 + 128],
                            QT8[32 * a:32 * a + 32, g, :, ss + o2:ss + o2 + w2],
                            start=True, stop=True, perf_mode=DR,
                            tile_position=(32 * a, 0))
                    ex = exb.tile([128, SH], BF16, tag="ex")
                    nc.scalar.activation(ex[:, :fd], sc[:, :fd],
                                         AF.Exp, scale=0.125)
                    if t0 >= s0:
                        nc.gpsimd.affine_select(
                            out=ex[:, 0:128], in_=ex[:, 0:128],
                            pattern=[[1, 128]],
                            compare_op=ALU.is_ge,
                            fill=0.0, base=0, channel_multiplier=-1)
                    for bi in range(nbank):
                        a2 = max(rel, 512 * bi)
                        b2 = min(SH, 512 * bi + 512)
                        if a2 >= b2:
                            continue
                        nc.tensor.matmul(
                            ctx[:, a2:b2], V[:, k, h, :],
                            ex[:, a2 - rel:b2 - rel],
                            start=(k == 0), stop=(k == last_k[bi]))
                    filler()
                # normalize: ctx[0:64] * (1/ctx[64]) -> ctxT slice
                rec = nrm.tile([1, SH], F32, tag="rec")
                nc.vector.reciprocal(rec[:], ctx[64:65, :])
                rbc = nrm.tile([64, SH], F32, tag="rbc")
                nc.gpsimd.partition_broadcast(rbc[:], rec[0:1, :])
                po = 64 * (h % 2)
                nc.vector.tensor_tensor(
                    ctxT[po:po + 64, h // 2, s0:s0 + SH],
                    ctx[0:64, :], rbc[:], ALU.mult)

            class Filler:
                """Dispenses queued work units evenly over `slots` calls."""
                def __init__(self, units, slots):
                    self.units = list(units)
                    self.slots = max(1, slots)
                    self.acc = 0.0
                    self.rate = len(self.units) / self.slots

                def __call__(self):
                    self.acc += self.rate
                    while self.acc >= 1.0 and self.units:
                        self.units.pop(0)()
                        self.acc -= 1.0

                def drain(self):
                    while self.units:
                        self.units.pop(0)()

            # ---- emission schedule ----------------------------------------
            # pieces 0..NP/2-1 up front (keys/queries for j=0)
            for p in range(NP // 2):
                for u in piece_units(p):
                    u()

            # j=0 attention, pieces NP/2..NP-1 as filler
            units_j0 = [u for p in range(NP // 2, NP) for u in piece_units(p)]
            slots_j0 = NHL * (SH // 128)
            f0 = Filler(units_j0, slots_j0)
            for h in range(NHL):
                attn_head(0, h, f0)
            f0.drain()

            # j=1 attention, out-proj for tokens of the first half as filler
            units_j1 = [
                (lambda si=si, o=o: unit_E(si, o))
                for si in range(NT // 2) for o in range(2)
            ]
            slots_j1 = NHL * (SH // 128) * 2
            f1 = Filler(units_j1, slots_j1)
            for h in range(NHL):
                attn_head(1, h, f1)
            f1.drain()

            # remaining out-proj tiles
            for si in range(NT // 2, NT):
                for o in range(2):
                    unit_E(si, o)

    nc.finalize()
    return nc


def _perm512():
    """Column permutation for w_kvu_k / w_qu so that PSUM chunk j, row
    32a+p corresponds to head 4*(j//2)+a, dim 32*(j%2)+p."""
    perm = np.empty(512, dtype=np.int64)
    for j in range(4):
        for a in range(4):
            for p in range(32):
                perm[128 * j + 32 * a + p] = 64 * (4 * (j // 2) + a) + 32 * (j % 2) + p
    return perm


def shard_inputs(inputs, S=2048):
    """Build the 8 per-core input maps from full inputs (host-side prep)."""
    f32 = lambda a: np.ascontiguousarray(np.asarray(a, dtype=np.float32))
    bf = lambda a: np.ascontiguousarray(
        np.asarray(a, dtype=np.float32).astype(ml_dtypes.bfloat16))
    x = f32(inputs["x"])
    w_kvc, b_kvc = f32(inputs["w_kvc"]), f32(inputs["b_kvc"])
    w_kvu, b_kvu = f32(inputs["w_kvu"]), f32(inputs["b_kvu"])
    w_qc, b_qc = f32(inputs["w_qc"]), f32(inputs["b_qc"])
    w_qu, b_qu = f32(inputs["w_qu"]), f32(inputs["b_qu"])
    w_o, b_o = f32(inputs["w_o"]), f32(inputs["b_o"])
    perm = _perm512()
    in_maps = []
    for core in range(NCORES):
        b = core // 2
        g2 = core % 2
        ks = slice(512 * g2, 512 * g2 + 512)            # K-feature slice
        vs = slice(DIM + 512 * g2, DIM + 512 * g2 + 512)  # V-feature slice
        in_maps.append({
            "x": bf(x[b].T.reshape(ND, 128, S)),
            "w_kvc": bf(w_kvc.reshape(ND, 128, LAT)),
            "w_qc": bf(w_qc.reshape(ND, 128, QR)),
            "w_kvu_k": bf(w_kvu[:, ks][:, perm]),
            "w_qu": bf(w_qu[:, ks][:, perm].reshape(2, 128, 512)),
            "w_kvu_v": bf(w_kvu[:, vs]),
            "w_o": bf(w_o[ks, :].reshape(4, 128, DIM)),
            "b_kvc": f32(b_kvc.reshape(LAT, 1)),
            "b_qc": f32(b_qc.reshape(2, 128).T),
            "b_qu": f32(b_qu[ks][perm].reshape(4, 128).T),
            "b_kvu_k": f32(b_kvu[ks][perm].reshape(4, 128).T),
            "b_kvu_v": f32(b_kvu[vs].reshape(1, 512)),
            "b_o": f32((b_o * 0.5).reshape(1, DIM)),
        })
    return in_maps


def kernel(**inputs) -> np.ndarray:
    from concourse.bass_utils import run_bass_kernel_spmd

    x = np.asarray(inputs["x"])
    S = x.shape[1]
    nc = build_mla(S=S)
    in_maps = shard_inputs(inputs, S=S)
    res = run_bass_kernel_spmd(nc, in_maps, list(range(NCORES))).results
    out = np.empty((B, S, DIM), dtype=np.float32)
    for b in range(B):
        out[b] = res[2 * b]["out"] + res[2 * b + 1]["out"]
    return out


# revision 7
# speedup vs baseline: 1.2809x; 1.0340x over previous
"""MLA (multi-head latent attention) Bass kernel for Trainium2, 8 NeuronCores.

Sharding: core i handles batch b = i // 2 and head-group g2 = i % 2
(8 of the 16 heads).  Each core computes a partial output
(its heads' contribution through out_proj, plus b_o/2); the host sums
the two partials per batch.

v2 design (ACT-bound):
  - All heavy host-transformable data arrives pre-laid-out: x transposed
    to xT bf16 [8,128,2048], weights pre-cast bf16, K/Q up-projection
    columns pre-permuted so the fp8 DoubleRow layout falls out of plain
    PSUM evacuations.
  - QK^T runs in fp8e4 DoubleRow perf mode: KT8/QT8 are stored
    [128p, g, plane, S] where partition 32a+p / plane pl encodes head
    4g+a, dim 32*pl+p; one matmul contracts all 64 head dims (2 k-tiles
    of 32) at 0.5 cycles/col.
  - All other matmuls are bf16 (1 cycle/col, no small-output penalty).
  - Softmax exp on ScalarE is the per-core floor (~135us); the emission
    order software-pipelines projections (2nd half) and out-proj tiles
    into the attention loops so PE fills its slack while ACT grinds.
  - PSUM: scores [128,1024]x2bufs (4 banks) + ctx [65,1024] (2 banks)
    + shared [128,512]x2 work tiles for projections/out-proj (2 banks).
  - Bias adds ride the PSUM->SBUF evacuations (DVE tensor_scalar_add,
    Pool tensor_tensor) - no PE bias matmuls.
"""

import numpy as np
import ml_dtypes

import concourse.bass as bass
import concourse.bacc as bacc
import concourse.mybir as mybir
import concourse.tile as tile

DIM = 1024
NUM_HEADS = 16
HEAD_DIM = 64
LAT = 128
QR = 256
B = 4
NCORES = 8
ND = DIM // 128       # 8 d-chunks
NHL = 8               # heads per core
F32 = mybir.dt.float32
BF16 = mybir.dt.bfloat16
FP8 = mybir.dt.float8e4
AF = mybir.ActivationFunctionType
ALU = mybir.AluOpType
DR = mybir.MatmulPerfMode.DoubleRow


def _pieces(total, w=512):
    return [(o, min(w, total - o)) for o in range(0, total, w)]


def build_mla(S=2048):
    """Build the per-core Bass program (same SPMD program on all 8 cores)."""
    assert S % 1024 == 0
    SH = S // 2           # s-half width
    NT = S // 128         # number of 128-token chunks
    NP = S // 512         # number of 512-token pieces

    nc = bacc.Bacc()

    x_d = nc.declare_dram_parameter("x", [ND, 128, S], BF16, isOutput=False)
    w_kvc_d = nc.declare_dram_parameter("w_kvc", [ND, 128, LAT], BF16, isOutput=False)
    w_qc_d = nc.declare_dram_parameter("w_qc", [ND, 128, QR], BF16, isOutput=False)
    w_kvu_k_d = nc.declare_dram_parameter("w_kvu_k", [128, 512], BF16, isOutput=False)
    w_qu_d = nc.declare_dram_parameter("w_qu", [2, 128, 512], BF16, isOutput=False)
    w_kvu_v_d = nc.declare_dram_parameter("w_kvu_v", [128, 512], BF16, isOutput=False)
    w_o_d = nc.declare_dram_parameter("w_o", [4, 128, DIM], BF16, isOutput=False)
    b_kvc_d = nc.declare_dram_parameter("b_kvc", [LAT, 1], F32, isOutput=False)
    b_qc_d = nc.declare_dram_parameter("b_qc", [128, 2], F32, isOutput=False)
    b_qu_d = nc.declare_dram_parameter("b_qu", [128, 4], F32, isOutput=False)
    b_kvu_k_d = nc.declare_dram_parameter("b_kvu_k", [128, 4], F32, isOutput=False)
    b_kvu_v_d = nc.declare_dram_parameter("b_kvu_v", [1, 512], F32, isOutput=False)
    out_d = nc.declare_dram_parameter("out", [S, DIM], F32, isOutput=True)

    with tile.TileContext(nc) as tc:
        with (
            tc.tile_pool(name="wts", bufs=1) as wts,
            tc.tile_pool(name="big", bufs=1) as big,
            tc.tile_pool(name="lat", bufs=2) as latp,
            tc.tile_pool(name="exb", bufs=4) as exb,
            tc.tile_pool(name="nrm", bufs=2) as nrm,
            tc.tile_pool(name="obp", bufs=4) as obp,
            tc.tile_pool(name="psc", bufs=2, space="PSUM") as psc,
            tc.tile_pool(name="pctx", bufs=1, space="PSUM") as pctx,
            tc.tile_pool(name="pwk", bufs=2, space="PSUM") as pwk,
        ):
            # ---- xT first on the SP queue (piece-major so piece 0 is
            # ready after 8 descriptors); weights ride the idle ACT queue ---
            xT = big.tile([128, ND, S], BF16, name="xT")
            for p in range(NP):
                for dc in range(ND):
                    nc.sync.dma_start(
                        out=xT[:, dc, 512 * p:512 * p + 512],
                        in_=x_d[dc, :, 512 * p:512 * p + 512])

            # ---- weights straight into SBUF (pre-cast on host) -------------
            w_kvc_sb = wts.tile([128, ND, LAT], BF16, name="w_kvc_sb")
            w_qc_sb = wts.tile([128, ND, QR], BF16, name="w_qc_sb")
            for dc in range(ND):
                nc.scalar.dma_start(out=w_kvc_sb[:, dc, :], in_=w_kvc_d[dc, :, :])
                nc.scalar.dma_start(out=w_qc_sb[:, dc, :], in_=w_qc_d[dc, :, :])
            w_kvu_k_sb = wts.tile([128, 512], BF16, name="w_kvu_k_sb")
            nc.scalar.dma_start(out=w_kvu_k_sb[:], in_=w_kvu_k_d[:, :])
            w_qu_sb = wts.tile([128, 2, 512], BF16, name="w_qu_sb")
            for qh in range(2):
                nc.scalar.dma_start(out=w_qu_sb[:, qh, :], in_=w_qu_d[qh, :, :])
            w_kvu_v_sb = wts.tile([128, 512], BF16, name="w_kvu_v_sb")
            nc.scalar.dma_start(out=w_kvu_v_sb[:], in_=w_kvu_v_d[:, :])
            w_o_sb = wts.tile([128, 4, DIM], BF16, name="w_o_sb")
            for cc in range(4):
                nc.scalar.dma_start(out=w_o_sb[:, cc, :], in_=w_o_d[cc, :, :])

            b_kvc_sb = wts.tile([128, 1], F32, name="b_kvc_sb")
            nc.scalar.dma_start(out=b_kvc_sb[:], in_=b_kvc_d[:, :])
            b_qc_sb = wts.tile([128, 2], F32, name="b_qc_sb")
            nc.scalar.dma_start(out=b_qc_sb[:], in_=b_qc_d[:, :])
            b_qu_sb = wts.tile([128, 4], F32, name="b_qu_sb")
            nc.scalar.dma_start(out=b_qu_sb[:], in_=b_qu_d[:, :])
            b_kvu_k_sb = wts.tile([128, 4], F32, name="b_kvu_k_sb")
            nc.scalar.dma_start(out=b_kvu_k_sb[:], in_=b_kvu_k_d[:, :])
            bv_row = wts.tile([1, 512], F32, name="bv_row")
            nc.scalar.dma_start(out=bv_row[:], in_=b_kvu_v_d[:, :])
            bvb = wts.tile([128, 512], F32, name="bvb")
            nc.gpsimd.partition_broadcast(bvb[:], bv_row[0:1, :])

            # ---- persistent tensors ---------------------------------------
            # KT8/QT8: [128p, g, plane, S]; partition 32a+p, plane pl
            # holds head 4g+a, dim 32*pl+p (fp8 for DoubleRow QK).
            KT8 = big.tile([128, 2, 2, S], FP8, name="KT8")
            QT8 = big.tile([128, 2, 2, S], FP8, name="QT8")
            # V: [128tok, chunk, head, 65] (64 vals + ones col)
            V = big.tile([128, NT, NHL, 65], BF16, name="V")
            nc.gpsimd.memset(V[:, :, :, 64:65], 1.0)
            # ctxT: [128 (2 heads x 64 dims), chunk h//2, S]
            ctxT = big.tile([128, 4, S], BF16, name="ctxT")

            # ---- work-unit emitters ---------------------------------------
            def unit_kv(p):
                off = 512 * p
                kvp = pwk.tile([128, 512], F32, tag="wk")
                for dc in range(ND):
                    nc.tensor.matmul(
                        kvp[:], w_kvc_sb[:, dc, :],
                        xT[:, dc, off:off + 512],
                        start=(dc == 0), stop=(dc == ND - 1))
                kvs = latp.tile([128, 512], BF16, tag=f"kvs")
                nc.vector.tensor_scalar_add(kvs[:], kvp[:], b_kvc_sb[:, 0:1])
                return kvs

            def unit_q(p, qh):
                off = 512 * p
                qp = pwk.tile([128, 512], F32, tag="wk")
                for dc in range(ND):
                    nc.tensor.matmul(
                        qp[:], w_qc_sb[:, dc, 128 * qh:128 * qh + 128],
                        xT[:, dc, off:off + 512],
                        start=(dc == 0), stop=(dc == ND - 1))
                qs = latp.tile([128, 512], BF16, tag=f"q{qh}s")
                nc.vector.tensor_scalar_add(qs[:], qp[:], b_qc_sb[:, qh:qh + 1])
                return qs

            def unit_KT(p, j, kvs):
                off = 512 * p
                kp = pwk.tile([128, 512], F32, tag="wk")
                nc.tensor.matmul(kp[:], w_kvu_k_sb[:, 128 * j:128 * j + 128],
                                 kvs[:], start=True, stop=True)
                nc.vector.tensor_scalar_add(
                    KT8[:, j // 2, j % 2, off:off + 512], kp[:],
                    b_kvu_k_sb[:, j:j + 1])

            def unit_QT(p, j, q0s, q1s):
                off = 512 * p
                qp = pwk.tile([128, 512], F32, tag="wk")
                nc.tensor.matmul(qp[:], w_qu_sb[:, 0, 128 * j:128 * j + 128],
                                 q0s[:], start=True, stop=False)
                nc.tensor.matmul(qp[:], w_qu_sb[:, 1, 128 * j:128 * j + 128],
                                 q1s[:], start=False, stop=True)
                nc.vector.tensor_scalar_add(
                    QT8[:, j // 2, j % 2, off:off + 512], qp[:],
                    b_qu_sb[:, j:j + 1])

            def unit_V(p, q, kvs):
                k = 4 * p + q
                vp = pwk.tile([128, 512], F32, tag="wk")
                nc.tensor.matmul(vp[:], kvs[:, 128 * q:128 * q + 128],
                                 w_kvu_v_sb[:], start=True, stop=True)
                nc.vector.tensor_tensor(
                    V[:, k, :, 0:64],
                    vp[:].rearrange("p (h c) -> p h c", c=64),
                    bvb[:].rearrange("p (h c) -> p h c", c=64), ALU.add)

            def piece_units(p):
                state = {}
                yield lambda: state.__setitem__("kvs", unit_kv(p))
                yield lambda: state.__setitem__("q0s", unit_q(p, 0))
                yield lambda: state.__setitem__("q1s", unit_q(p, 1))
                for j in range(4):
                    yield lambda j=j: unit_KT(p, j, state["kvs"])
                for j in range(4):
                    yield lambda j=j: unit_QT(p, j, state["q0s"], state["q1s"])
                for q in range(4):
                    yield lambda q=q: unit_V(p, q, state["kvs"])

            def unit_E(si, o):
                op = pwk.tile([128, 512], F32, tag="wk")
                for cc in range(4):
                    nc.tensor.matmul(
                        op[:], ctxT[:, cc, 128 * si:128 * si + 128],
                        w_o_sb[:, cc, 512 * o:512 * o + 512],
                        start=(cc == 0), stop=(cc == 3))
                ob = obp.tile([128, 512], F32, tag="ob")
                nc.vector.tensor_copy(ob[:], op[:])
                nc.sync.dma_start(
                    out=out_d[128 * si:128 * si + 128, 512 * o:512 * o + 512],
                    in_=ob[:])

            def attn_head(j, h, filler):
                """Attention for s-half j, local head h; pulls filler work
                after each key chunk to keep PE fed while ACT runs exp."""
                s0 = SH * j
                kmax = (SH // 128) * (j + 1)
                nbank = SH // 512
                last_k = {
                    bi: min(kmax - 1, (s0 + 512 * (bi + 1)) // 128 - 1)
                    for bi in range(nbank)
                }
                g, a = h // 4, h % 4
                ctx = pctx.tile([65, SH], F32, tag="ctx")
                for k in range(kmax):
                    t0 = 128 * k
                    ss = max(s0, t0)
                    fd = s0 + SH - ss
                    rel = ss - s0
                    sc = psc.tile([128, SH], F32, tag="sc")
                    for o2, w2 in _pieces(fd, 256):
                        nc.tensor.matmul(
                            sc[:, o2:o2 + w2],
                            KT8[32 * a:32 * a + 32, g, :, t0:t0 + 128],
                            QT8[32 * a:32 * a + 32, g, :, ss + o2:ss + o2 + w2],
                            start=True, stop=True, perf_mode=DR,
                            tile_position=(32 * a, 0))
                    ex = exb.tile([128, SH], BF16, tag="ex")
                    nc.scalar.activation(ex[:, :fd], sc[:, :fd],
                                         AF.Exp, scale=0.125)
                    if t0 >= s0:
                        nc.gpsimd.affine_select(
                            out=ex[:, 0:128], in_=ex[:, 0:128],
                            pattern=[[1, 128]],
                            compare_op=ALU.is_ge,
                            fill=0.0, base=0, channel_multiplier=-1)
                    for bi in range(nbank):
                        a2 = max(rel, 512 * bi)
                        b2 = min(SH, 512 * bi + 512)
                        if a2 >= b2:
                            continue
                        nc.tensor.matmul(
                            ctx[:, a2:b2], V[:, k, h, :],
                            ex[:, a2 - rel:b2 - rel],
                            start=(k == 0), stop=(k == last_k[bi]))
                    filler()
                # normalize: ctx[0:64] * (1/ctx[64]) -> ctxT slice
                rec = nrm.tile([1, SH], F32, tag="rec")
                nc.vector.reciprocal(rec[:], ctx[64:65, :])
                rbc = nrm.tile([64, SH], F32, tag="rbc")
                nc.gpsimd.partition_broadcast(rbc[:], rec[0:1, :])
                po = 64 * (h % 2)
                nc.vector.tensor_tensor(
                    ctxT[po:po + 64, h // 2, s0:s0 + SH],
                    ctx[0:64, :], rbc[:], ALU.mult)

            class Filler:
                """Dispenses queued work units evenly over `slots` calls."""
                def __init__(self, units, slots):
                    self.units = list(units)
                    self.slots = max(1, slots)
                    self.acc = 0.0
                    self.rate = len(self.units) / self.slots

                def __call__(self):
                    self.acc += self.rate
                    while self.acc >= 1.0 and self.units:
                        self.units.pop(0)()
                        self.acc -= 1.0

                def drain(self):
                    while self.units:
                        self.units.pop(0)()

            # ---- emission schedule ----------------------------------------
            # pieces 0..NP/2-1 up front (keys/queries for j=0)
            for p in range(NP // 2):
                for u in piece_units(p):
                    u()

            # j=0 attention, pieces NP/2..NP-1 as filler
            units_j0 = [u for p in range(NP // 2, NP) for u in piece_units(p)]
            slots_j0 = NHL * (SH // 128)
            f0 = Filler(units_j0, slots_j0)
            for h in range(NHL):
                attn_head(0, h, f0)
            f0.drain()

            # j=1 attention, out-proj for tokens of the first half as filler
            units_j1 = [
                (lambda si=si, o=o: unit_E(si, o))
                for si in range(NT // 2) for o in range(2)
            ]
            slots_j1 = NHL * (SH // 128) * 2
            f1 = Filler(units_j1, slots_j1)
            for h in range(NHL):
                attn_head(1, h, f1)
            f1.drain()

            # remaining out-proj tiles
            for si in range(NT // 2, NT):
                for o in range(2):
                    unit_E(si, o)

    nc.finalize()
    return nc


def _perm512():
    """Column permutation for w_kvu_k / w_qu so that PSUM chunk j, row
    32a+p corresponds to head 4*(j//2)+a, dim 32*(j%2)+p."""
    perm = np.empty(512, dtype=np.int64)
    for j in range(4):
        for a in range(4):
            for p in range(32):
                perm[128 * j + 32 * a + p] = 64 * (4 * (j // 2) + a) + 32 * (j % 2) + p
    return perm


def shard_inputs(inputs, S=2048):
    """Build the 8 per-core input maps from full inputs (host-side prep)."""
    f32 = lambda a: np.ascontiguousarray(np.asarray(a, dtype=np.float32))
    bf = lambda a: np.ascontiguousarray(
        np.asarray(a, dtype=np.float32).astype(ml_dtypes.bfloat16))
    x = f32(inputs["x"])
    w_kvc, b_kvc = f32(inputs["w_kvc"]), f32(inputs["b_kvc"])
    w_kvu, b_kvu = f32(inputs["w_kvu"]), f32(inputs["b_kvu"])
    w_qc, b_qc = f32(inputs["w_qc"]), f32(inputs["b_qc"])
    w_qu, b_qu = f32(inputs["w_qu"]), f32(inputs["b_qu"])
    w_o, b_o = f32(inputs["w_o"]), f32(inputs["b_o"])
    perm = _perm512()
    in_maps = []
    for core in range(NCORES):
        b = core // 2
        g2 = core % 2
        ks = slice(512 * g2, 512 * g2 + 512)            # K-feature slice
        vs = slice(DIM + 512 * g2, DIM + 512 * g2 + 512)  # V-feature slice
        in_maps.append({
            "x": bf(x[b].T.reshape(ND, 128, S)),
            "w_kvc": bf(w_kvc.reshape(ND, 128, LAT)),
            "w_qc": bf(w_qc.reshape(ND, 128, QR)),
            "w_kvu_k": bf(w_kvu[:, ks][:, perm]),
            "w_qu": bf(w_qu[:, ks][:, perm].reshape(2, 128, 512)),
            "w_kvu_v": bf(w_kvu[:, vs]),
            "w_o": bf(w_o[ks, :].reshape(4, 128, DIM)),
            "b_kvc": f32(b_kvc.reshape(LAT, 1)),
            "b_qc": f32(b_qc.reshape(2, 128).T),
            "b_qu": f32(b_qu[ks][perm].reshape(4, 128).T),
            "b_kvu_k": f32(b_kvu[ks][perm].reshape(4, 128).T),
            "b_kvu_v": f32(b_kvu[vs].reshape(1, 512)),
        })
    return in_maps


def kernel(**inputs) -> np.ndarray:
    from concourse.bass_utils import run_bass_kernel_spmd

    x = np.asarray(inputs["x"])
    S = x.shape[1]
    nc = build_mla(S=S)
    in_maps = shard_inputs(inputs, S=S)
    res = run_bass_kernel_spmd(nc, in_maps, list(range(NCORES))).results
    b_o = np.asarray(inputs["b_o"], dtype=np.float32)
    out = np.empty((B, S, DIM), dtype=np.float32)
    for b in range(B):
        out[b] = res[2 * b]["out"] + res[2 * b + 1]["out"] + b_o
    return out


# revision 9
# speedup vs baseline: 1.3345x; 1.0418x over previous
"""MLA (multi-head latent attention) Bass kernel for Trainium2, 8 NeuronCores.

Sharding: core i handles batch b = i // 2 and head-group g2 = i % 2
(8 of the 16 heads).  Each core computes a partial output
(its heads' contribution through out_proj, plus b_o/2); the host sums
the two partials per batch.

v2 design (ACT-bound):
  - All heavy host-transformable data arrives pre-laid-out: x transposed
    to xT bf16 [8,128,2048], weights pre-cast bf16, K/Q up-projection
    columns pre-permuted so the fp8 DoubleRow layout falls out of plain
    PSUM evacuations.
  - QK^T runs in fp8e4 DoubleRow perf mode: KT8/QT8 are stored
    [128p, g, plane, S] where partition 32a+p / plane pl encodes head
    4g+a, dim 32*pl+p; one matmul contracts all 64 head dims (2 k-tiles
    of 32) at 0.5 cycles/col.
  - All other matmuls are bf16 (1 cycle/col, no small-output penalty).
  - Softmax exp on ScalarE is the per-core floor (~135us); the emission
    order software-pipelines projections (2nd half) and out-proj tiles
    into the attention loops so PE fills its slack while ACT grinds.
  - PSUM: scores [128,1024]x2bufs (4 banks) + ctx [65,1024] (2 banks)
    + shared [128,512]x2 work tiles for projections/out-proj (2 banks).
  - Bias adds ride the PSUM->SBUF evacuations (DVE tensor_scalar_add,
    Pool tensor_tensor) - no PE bias matmuls.
"""

import numpy as np
import ml_dtypes

import concourse.bass as bass
import concourse.bacc as bacc
import concourse.mybir as mybir
import concourse.tile as tile

DIM = 1024
NUM_HEADS = 16
HEAD_DIM = 64
LAT = 128
QR = 256
B = 4
NCORES = 8
ND = DIM // 128       # 8 d-chunks
NHL = 8               # heads per core
F32 = mybir.dt.float32
BF16 = mybir.dt.bfloat16
FP8 = mybir.dt.float8e4
AF = mybir.ActivationFunctionType
ALU = mybir.AluOpType
DR = mybir.MatmulPerfMode.DoubleRow


def _pieces(total, w=512):
    return [(o, min(w, total - o)) for o in range(0, total, w)]


def build_mla(S=2048):
    """Build the per-core Bass program (same SPMD program on all 8 cores)."""
    assert S % 1024 == 0
    SH = S // 2           # s-half width
    NT = S // 128         # number of 128-token chunks
    NP = S // 512         # number of 512-token pieces

    nc = bacc.Bacc()

    x_d = nc.declare_dram_parameter("x", [ND, 128, S], BF16, isOutput=False)
    w_kvc_d = nc.declare_dram_parameter("w_kvc", [ND, 128, LAT], BF16, isOutput=False)
    x8_d = nc.declare_dram_parameter("x8", [64, ND, 2, S], FP8, isOutput=False)
    w_qc8_d = nc.declare_dram_parameter("w_qc8", [64, ND, 2, QR], FP8, isOutput=False)
    w_kvu_k_d = nc.declare_dram_parameter("w_kvu_k", [128, 512], BF16, isOutput=False)
    w_qu8_d = nc.declare_dram_parameter("w_qu8", [128, 2, 512], FP8, isOutput=False)
    w_kvu_v_d = nc.declare_dram_parameter("w_kvu_v", [128, 512], BF16, isOutput=False)
    w_o_d = nc.declare_dram_parameter("w_o", [4, 128, DIM], BF16, isOutput=False)
    b_kvc_d = nc.declare_dram_parameter("b_kvc", [LAT, 1], F32, isOutput=False)
    b_qc_d = nc.declare_dram_parameter("b_qc", [128, 2], F32, isOutput=False)
    b_qu_d = nc.declare_dram_parameter("b_qu", [128, 4], F32, isOutput=False)
    b_kvu_k_d = nc.declare_dram_parameter("b_kvu_k", [128, 4], F32, isOutput=False)
    b_kvu_v_d = nc.declare_dram_parameter("b_kvu_v", [1, 512], F32, isOutput=False)
    out_d = nc.declare_dram_parameter("out", [S, DIM], F32, isOutput=True)

    with tile.TileContext(nc) as tc:
        with (
            tc.tile_pool(name="wts", bufs=1) as wts,
            tc.tile_pool(name="big", bufs=1) as big,
            tc.tile_pool(name="lat", bufs=2) as latp,
            tc.tile_pool(name="exb", bufs=4) as exb,
            tc.tile_pool(name="nrm", bufs=2) as nrm,
            tc.tile_pool(name="obp", bufs=4) as obp,
            tc.tile_pool(name="psc", bufs=2, space="PSUM") as psc,
            tc.tile_pool(name="pctx", bufs=1, space="PSUM") as pctx,
            tc.tile_pool(name="pwk", bufs=2, space="PSUM") as pwk,
        ):
            # ---- xT first on the SP queue (piece-major so piece 0 is
            # ready after 8 descriptors); weights ride the idle ACT queue ---
            xT = big.tile([128, ND, S], BF16, name="xT")
            x8T = big.tile([64, ND, 2, S], FP8, name="x8T")
            for p in range(NP):
                for dc in range(ND):
                    nc.sync.dma_start(
                        out=xT[:, dc, 512 * p:512 * p + 512],
                        in_=x_d[dc, :, 512 * p:512 * p + 512])
                nc.sync.dma_start(
                    out=x8T[:, :, :, 512 * p:512 * p + 512],
                    in_=x8_d[:, :, :, 512 * p:512 * p + 512])

            # ---- weights straight into SBUF (pre-cast on host) -------------
            w_kvc_sb = wts.tile([128, ND, LAT], BF16, name="w_kvc_sb")
            for dc in range(ND):
                nc.scalar.dma_start(out=w_kvc_sb[:, dc, :], in_=w_kvc_d[dc, :, :])
            w_qc8_sb = wts.tile([64, ND, 2, QR], FP8, name="w_qc8_sb")
            nc.scalar.dma_start(out=w_qc8_sb[:], in_=w_qc8_d[:, :, :, :])
            w_kvu_k_sb = wts.tile([128, 512], BF16, name="w_kvu_k_sb")
            nc.scalar.dma_start(out=w_kvu_k_sb[:], in_=w_kvu_k_d[:, :])
            w_qu8_sb = wts.tile([128, 2, 512], FP8, name="w_qu8_sb")
            nc.scalar.dma_start(out=w_qu8_sb[:], in_=w_qu8_d[:, :, :])
            w_kvu_v_sb = wts.tile([128, 512], BF16, name="w_kvu_v_sb")
            nc.scalar.dma_start(out=w_kvu_v_sb[:], in_=w_kvu_v_d[:, :])
            w_o_sb = wts.tile([128, 4, DIM], BF16, name="w_o_sb")
            for cc in range(4):
                nc.scalar.dma_start(out=w_o_sb[:, cc, :], in_=w_o_d[cc, :, :])

            b_kvc_sb = wts.tile([128, 1], F32, name="b_kvc_sb")
            nc.scalar.dma_start(out=b_kvc_sb[:], in_=b_kvc_d[:, :])
            b_qc_sb = wts.tile([128, 2], F32, name="b_qc_sb")
            nc.scalar.dma_start(out=b_qc_sb[:], in_=b_qc_d[:, :])
            b_qu_sb = wts.tile([128, 4], F32, name="b_qu_sb")
            nc.scalar.dma_start(out=b_qu_sb[:], in_=b_qu_d[:, :])
            b_kvu_k_sb = wts.tile([128, 4], F32, name="b_kvu_k_sb")
            nc.scalar.dma_start(out=b_kvu_k_sb[:], in_=b_kvu_k_d[:, :])
            bv_row = wts.tile([1, 512], F32, name="bv_row")
            nc.scalar.dma_start(out=bv_row[:], in_=b_kvu_v_d[:, :])
            bvb = wts.tile([128, 512], F32, name="bvb")
            nc.gpsimd.partition_broadcast(bvb[:], bv_row[0:1, :])

            # ---- persistent tensors ---------------------------------------
            # KT8/QT8: [128p, g, plane, S]; partition 32a+p, plane pl
            # holds head 4g+a, dim 32*pl+p (fp8 for DoubleRow QK).
            KT8 = big.tile([128, 2, 2, S], FP8, name="KT8")
            QT8 = big.tile([128, 2, 2, S], FP8, name="QT8")
            # V: [128tok, chunk, head, 65] (64 vals + ones col)
            V = big.tile([128, NT, NHL, 65], BF16, name="V")
            nc.gpsimd.memset(V[:, :, :, 64:65], 1.0)
            # ctxT: [128 (2 heads x 64 dims), chunk h//2, S]
            ctxT = big.tile([128, 4, S], BF16, name="ctxT")

            # ---- work-unit emitters ---------------------------------------
            def unit_kv(p):
                off = 512 * p
                kvp = pwk.tile([128, 512], F32, tag="wk")
                for dc in range(ND):
                    nc.tensor.matmul(
                        kvp[:], w_kvc_sb[:, dc, :],
                        xT[:, dc, off:off + 512],
                        start=(dc == 0), stop=(dc == ND - 1))
                kvs = latp.tile([128, 512], BF16, tag=f"kvs")
                nc.vector.tensor_scalar_add(kvs[:], kvp[:], b_kvc_sb[:, 0:1])
                return kvs

            def unit_q(p, qh, q8):
                off = 512 * p
                qp = pwk.tile([128, 512], F32, tag="wk")
                for o in (0, 256):
                    for dc in range(ND):
                        nc.tensor.matmul(
                            qp[:, o:o + 256],
                            w_qc8_sb[:, dc, :, 128 * qh:128 * qh + 128],
                            x8T[:, dc, :, off + o:off + o + 256],
                            start=(dc == 0), stop=(dc == ND - 1),
                            perf_mode=DR)
                nc.vector.tensor_scalar_add(q8[:, qh, :], qp[:],
                                            b_qc_sb[:, qh:qh + 1])

            def unit_KT(p, j, kvs):
                off = 512 * p
                kp = pwk.tile([128, 512], F32, tag="wk")
                nc.tensor.matmul(kp[:], w_kvu_k_sb[:, 128 * j:128 * j + 128],
                                 kvs[:], start=True, stop=True)
                nc.vector.tensor_scalar_add(
                    KT8[:, j // 2, j % 2, off:off + 512], kp[:],
                    b_kvu_k_sb[:, j:j + 1])

            def unit_QT(p, j, q8):
                off = 512 * p
                qp = pwk.tile([128, 512], F32, tag="wk")
                for o in (0, 256):
                    nc.tensor.matmul(
                        qp[:, o:o + 256], w_qu8_sb[:, :, 128 * j:128 * j + 128],
                        q8[:, :, o:o + 256],
                        start=True, stop=True, perf_mode=DR)
                nc.vector.tensor_scalar_add(
                    QT8[:, j // 2, j % 2, off:off + 512], qp[:],
                    b_qu_sb[:, j:j + 1])

            def unit_V(p, q, kvs):
                k = 4 * p + q
                vp = pwk.tile([128, 512], F32, tag="wk")
                nc.tensor.matmul(vp[:], kvs[:, 128 * q:128 * q + 128],
                                 w_kvu_v_sb[:], start=True, stop=True)
                nc.vector.tensor_tensor(
                    V[:, k, :, 0:64],
                    vp[:].rearrange("p (h c) -> p h c", c=64),
                    bvb[:].rearrange("p (h c) -> p h c", c=64), ALU.add)

            def piece_units(p):
                state = {}

                def mk_q8():
                    q8 = latp.tile([128, 2, 512], FP8, tag="q8")
                    state["q8"] = q8
                    unit_q(p, 0, q8)
                yield mk_q8
                yield lambda: unit_q(p, 1, state["q8"])
                yield lambda: state.__setitem__("kvs", unit_kv(p))
                for j in range(4):
                    yield lambda j=j: unit_QT(p, j, state["q8"])
                for j in range(4):
                    yield lambda j=j: unit_KT(p, j, state["kvs"])
                for q in range(4):
                    yield lambda q=q: unit_V(p, q, state["kvs"])

            def unit_E(si, o):
                op = pwk.tile([128, 512], F32, tag="wk")
                for cc in range(4):
                    nc.tensor.matmul(
                        op[:], ctxT[:, cc, 128 * si:128 * si + 128],
                        w_o_sb[:, cc, 512 * o:512 * o + 512],
                        start=(cc == 0), stop=(cc == 3))
                ob = obp.tile([128, 512], F32, tag="ob")
                nc.vector.tensor_copy(ob[:], op[:])
                nc.sync.dma_start(
                    out=out_d[128 * si:128 * si + 128, 512 * o:512 * o + 512],
                    in_=ob[:])

            def attn_head(j, h, filler):
                """Attention for s-half j, local head h; pulls filler work
                after each key chunk to keep PE fed while ACT runs exp."""
                s0 = SH * j
                kmax = (SH // 128) * (j + 1)
                nbank = SH // 512
                last_k = {
                    bi: min(kmax - 1, (s0 + 512 * (bi + 1)) // 128 - 1)
                    for bi in range(nbank)
                }
                g, a = h // 4, h % 4
                ctx = pctx.tile([65, SH], F32, tag="ctx")
                for k in range(kmax):
                    t0 = 128 * k
                    ss = max(s0, t0)
                    fd = s0 + SH - ss
                    rel = ss - s0
                    sc = psc.tile([128, SH], F32, tag="sc")
                    for o2, w2 in _pieces(fd, 256):
                        nc.tensor.matmul(
                            sc[:, o2:o2 + w2],
                            KT8[32 * a:32 * a + 32, g, :, t0:t0 + 128],
                            QT8[32 * a:32 * a + 32, g, :, ss + o2:ss + o2 + w2],
                            start=True, stop=True, perf_mode=DR,
                            tile_position=(32 * a, 0))
                    ex = exb.tile([128, SH], BF16, tag="ex")
                    nc.scalar.activation(ex[:, :fd], sc[:, :fd],
                                         AF.Exp, scale=0.125)
                    if t0 >= s0:
                        nc.gpsimd.affine_select(
                            out=ex[:, 0:128], in_=ex[:, 0:128],
                            pattern=[[1, 128]],
                            compare_op=ALU.is_ge,
                            fill=0.0, base=0, channel_multiplier=-1)
                    for bi in range(nbank):
                        a2 = max(rel, 512 * bi)
                        b2 = min(SH, 512 * bi + 512)
                        if a2 >= b2:
                            continue
                        nc.tensor.matmul(
                            ctx[:, a2:b2], V[:, k, h, :],
                            ex[:, a2 - rel:b2 - rel],
                            start=(k == 0), stop=(k == last_k[bi]))
                    filler()
                # normalize: ctx[0:64] * (1/ctx[64]) -> ctxT slice
                rec = nrm.tile([1, SH], F32, tag="rec")
                nc.vector.reciprocal(rec[:], ctx[64:65, :])
                rbc = nrm.tile([64, SH], F32, tag="rbc")
                nc.gpsimd.partition_broadcast(rbc[:], rec[0:1, :])
                po = 64 * (h % 2)
                nc.vector.tensor_tensor(
                    ctxT[po:po + 64, h // 2, s0:s0 + SH],
                    ctx[0:64, :], rbc[:], ALU.mult)

            class Filler:
                """Dispenses queued work units evenly over `slots` calls."""
                def __init__(self, units, slots):
                    self.units = list(units)
                    self.slots = max(1, slots)
                    self.acc = 0.0
                    self.rate = len(self.units) / self.slots

                def __call__(self):
                    self.acc += self.rate
                    while self.acc >= 1.0 and self.units:
                        self.units.pop(0)()
                        self.acc -= 1.0

                def drain(self):
                    while self.units:
                        self.units.pop(0)()

            # ---- emission schedule ----------------------------------------
            # pieces 0..NP/2-1 up front (keys/queries for j=0)
            for p in range(NP // 2):
                for u in piece_units(p):
                    u()

            # j=0 attention, pieces NP/2..NP-1 as filler
            units_j0 = [u for p in range(NP // 2, NP) for u in piece_units(p)]
            slots_j0 = NHL * (SH // 128)
            f0 = Filler(units_j0, slots_j0)
            for h in range(NHL):
                attn_head(0, h, f0)
            f0.drain()

            # j=1 attention, out-proj for tokens of the first half as filler
            units_j1 = [
                (lambda si=si, o=o: unit_E(si, o))
                for si in range(NT // 2) for o in range(2)
            ]
            slots_j1 = NHL * (SH // 128) * 2
            f1 = Filler(units_j1, slots_j1)
            for h in range(NHL):
                attn_head(1, h, f1)
            f1.drain()

            # remaining out-proj tiles
            for si in range(NT // 2, NT):
                for o in range(2):
                    unit_E(si, o)

    nc.finalize()
    return nc


def _perm512():
    """Column permutation for w_kvu_k / w_qu so that PSUM chunk j, row
    32a+p corresponds to head 4*(j//2)+a, dim 32*(j%2)+p."""
    perm = np.empty(512, dtype=np.int64)
    for j in range(4):
        for a in range(4):
            for p in range(32):
                perm[128 * j + 32 * a + p] = 64 * (4 * (j // 2) + a) + 32 * (j % 2) + p
    return perm


def shard_inputs(inputs, S=2048):
    """Build the 8 per-core input maps from full inputs (host-side prep)."""
    f32 = lambda a: np.ascontiguousarray(np.asarray(a, dtype=np.float32))
    bf = lambda a: np.ascontiguousarray(
        np.asarray(a, dtype=np.float32).astype(ml_dtypes.bfloat16))
    x = f32(inputs["x"])
    w_kvc, b_kvc = f32(inputs["w_kvc"]), f32(inputs["b_kvc"])
    w_kvu, b_kvu = f32(inputs["w_kvu"]), f32(inputs["b_kvu"])
    w_qc, b_qc = f32(inputs["w_qc"]), f32(inputs["b_qc"])
    w_qu, b_qu = f32(inputs["w_qu"]), f32(inputs["b_qu"])
    w_o, b_o = f32(inputs["w_o"]), f32(inputs["b_o"])
    perm = _perm512()
    fp8 = lambda a: np.ascontiguousarray(
        np.asarray(a, dtype=np.float32).astype(ml_dtypes.float8_e4m3))
    in_maps = []
    for core in range(NCORES):
        b = core // 2
        g2 = core % 2
        ks = slice(512 * g2, 512 * g2 + 512)            # K-feature slice
        vs = slice(DIM + 512 * g2, DIM + 512 * g2 + 512)  # V-feature slice
        in_maps.append({
            "x": bf(x[b].T.reshape(ND, 128, S)),
            "x8": fp8(x[b].T.reshape(ND, 2, 64, S).transpose(2, 0, 1, 3)),
            "w_kvc": bf(w_kvc.reshape(ND, 128, LAT)),
            "w_qc8": fp8(w_qc.reshape(ND, 2, 64, QR).transpose(2, 0, 1, 3)),
            "w_kvu_k": bf(w_kvu[:, ks][:, perm]),
            "w_qu8": fp8(w_qu[:, ks][:, perm].reshape(2, 128, 512).transpose(1, 0, 2)),
            "w_kvu_v": bf(w_kvu[:, vs]),
            "w_o": bf(w_o[ks, :].reshape(4, 128, DIM)),
            "b_kvc": f32(b_kvc.reshape(LAT, 1)),
            "b_qc": f32(b_qc.reshape(2, 128).T),
            "b_qu": f32(b_qu[ks][perm].reshape(4, 128).T),
            "b_kvu_k": f32(b_kvu[ks][perm].reshape(4, 128).T),
            "b_kvu_v": f32(b_kvu[vs].reshape(1, 512)),
        })
    return in_maps


def kernel(**inputs) -> np.ndarray:
    from concourse.bass_utils import run_bass_kernel_spmd

    x = np.asarray(inputs["x"])
    S = x.shape[1]
    nc = build_mla(S=S)
    in_maps = shard_inputs(inputs, S=S)
    res = run_bass_kernel_spmd(nc, in_maps, list(range(NCORES))).results
    b_o = np.asarray(inputs["b_o"], dtype=np.float32)
    out = np.empty((B, S, DIM), dtype=np.float32)
    for b in range(B):
        out[b] = res[2 * b]["out"] + res[2 * b + 1]["out"] + b_o
    return out


# revision 10
# speedup vs baseline: 1.4011x; 1.0499x over previous
"""MLA (multi-head latent attention) Bass kernel for Trainium2, 8 NeuronCores.

Sharding: core i handles batch b = i // 2 and head-group g2 = i % 2
(8 of the 16 heads).  Each core computes a partial output
(its heads' contribution through out_proj); the host sums the two
partials per batch and adds b_o.

Design (ACT-bound; softmax exp on ScalarE is the per-core floor):
  - Host pre-lays-out everything: x transposed to bf16 xT [128,8,S] and
    fp8 x8T [64,8,2,S]; weights pre-cast (bf16 / fp8), K/Q up-projection
    columns pre-permuted so the fp8 DoubleRow layout falls out of plain
    PSUM evacuations.
  - QK^T runs in fp8e4 DoubleRow: KT8/QT8 stored [128p, g, plane, S]
    (partition 32a+p, plane pl = head 4g+a, dim 32pl+p); one matmul
    contracts all 64 head dims at 0.5 cycles/col.  The whole Q path
    (x->q_lat->QT) is fp8 DoubleRow too - it only feeds softmax scores,
    which tolerate fp8 noise.  V/out paths stay bf16.
  - Emission order software-pipelines: pieces 0-1 up front (deep scoped
    PSUM pool, KT/QT evacuations on the then-idle ACT engine), pieces
    2-3 as fillers inside j=0 attention, out-proj of the first token
    half as fillers inside j=1, remainder in a deep-pool tail with ACT
    evacuations.
  - PSUM: attention = scores [128,1024]x2bufs (4 banks) + ctx [65,1024]
    (2) + filler work tiles [128,512]x2 (2).
"""

import numpy as np
import ml_dtypes

import concourse.bass as bass
import concourse.bacc as bacc
import concourse.mybir as mybir
import concourse.tile as tile

DIM = 1024
NUM_HEADS = 16
HEAD_DIM = 64
LAT = 128
QR = 256
B = 4
NCORES = 8
ND = DIM // 128       # 8 d-chunks
NHL = 8               # heads per core
F32 = mybir.dt.float32
BF16 = mybir.dt.bfloat16
FP8 = mybir.dt.float8e4
AF = mybir.ActivationFunctionType
ALU = mybir.AluOpType
DR = mybir.MatmulPerfMode.DoubleRow


def _pieces(total, w=512):
    return [(o, min(w, total - o)) for o in range(0, total, w)]


def build_mla(S=2048):
    """Build the per-core Bass program (same SPMD program on all 8 cores)."""
    assert S % 1024 == 0
    SH = S // 2           # s-half width
    NT = S // 128         # number of 128-token chunks
    NP = S // 512         # number of 512-token pieces

    nc = bacc.Bacc()

    x_d = nc.declare_dram_parameter("x", [128, ND, S], BF16, isOutput=False)
    x8_d = nc.declare_dram_parameter("x8", [64, ND, 2, S], FP8, isOutput=False)
    w_kvc_d = nc.declare_dram_parameter("w_kvc", [128, ND, LAT], BF16, isOutput=False)
    w_qc8_d = nc.declare_dram_parameter("w_qc8", [64, ND, 2, QR], FP8, isOutput=False)
    w_kvu_k_d = nc.declare_dram_parameter("w_kvu_k", [128, 512], BF16, isOutput=False)
    w_qu8_d = nc.declare_dram_parameter("w_qu8", [128, 2, 512], FP8, isOutput=False)
    w_kvu_v_d = nc.declare_dram_parameter("w_kvu_v", [128, 512], BF16, isOutput=False)
    w_o_d = nc.declare_dram_parameter("w_o", [128, 4, DIM], BF16, isOutput=False)
    b_kvc_d = nc.declare_dram_parameter("b_kvc", [LAT, 1], F32, isOutput=False)
    b_qc_d = nc.declare_dram_parameter("b_qc", [128, 2], F32, isOutput=False)
    b_qu_d = nc.declare_dram_parameter("b_qu", [128, 4], F32, isOutput=False)
    b_kvu_k_d = nc.declare_dram_parameter("b_kvu_k", [128, 4], F32, isOutput=False)
    b_kvu_v_d = nc.declare_dram_parameter("b_kvu_v", [1, 512], F32, isOutput=False)
    out_d = nc.declare_dram_parameter("out", [S, DIM], F32, isOutput=True)

    with tile.TileContext(nc) as tc:
        with (
            tc.tile_pool(name="wts", bufs=1) as wts,
            tc.tile_pool(name="big", bufs=1) as big,
            tc.tile_pool(name="lat", bufs=2) as latp,
            tc.tile_pool(name="exb", bufs=4) as exb,
            tc.tile_pool(name="nrm", bufs=2) as nrm,
            tc.tile_pool(name="obp", bufs=4) as obp,
        ):
            # ---- early ACT-queue DMAs (small biases + proj weights) -------
            b_kvc_sb = wts.tile([128, 1], F32, name="b_kvc_sb")
            nc.scalar.dma_start(out=b_kvc_sb[:], in_=b_kvc_d[:, :])
            b_qc_sb = wts.tile([128, 2], F32, name="b_qc_sb")
            nc.scalar.dma_start(out=b_qc_sb[:], in_=b_qc_d[:, :])
            b_qu_sb = wts.tile([128, 4], F32, name="b_qu_sb")
            nc.scalar.dma_start(out=b_qu_sb[:], in_=b_qu_d[:, :])
            b_kvu_k_sb = wts.tile([128, 4], F32, name="b_kvu_k_sb")
            nc.scalar.dma_start(out=b_kvu_k_sb[:], in_=b_kvu_k_d[:, :])
            bv_row = wts.tile([1, 512], F32, name="bv_row")
            nc.scalar.dma_start(out=bv_row[:], in_=b_kvu_v_d[:, :])
            bvb = wts.tile([128, 512], F32, name="bvb")
            nc.gpsimd.partition_broadcast(bvb[:], bv_row[0:1, :])
            w_kvc_sb = wts.tile([128, ND, LAT], BF16, name="w_kvc_sb")
            nc.scalar.dma_start(out=w_kvc_sb[:], in_=w_kvc_d[:, :, :])
            w_qc8_sb = wts.tile([64, ND, 2, QR], FP8, name="w_qc8_sb")
            nc.scalar.dma_start(out=w_qc8_sb[:], in_=w_qc8_d[:, :, :, :])
            w_qu8_sb = wts.tile([128, 2, 512], FP8, name="w_qu8_sb")
            nc.scalar.dma_start(out=w_qu8_sb[:], in_=w_qu8_d[:, :, :])
            w_kvu_k_sb = wts.tile([128, 512], BF16, name="w_kvu_k_sb")
            nc.scalar.dma_start(out=w_kvu_k_sb[:], in_=w_kvu_k_d[:, :])
            w_kvu_v_sb = wts.tile([128, 512], BF16, name="w_kvu_v_sb")
            nc.scalar.dma_start(out=w_kvu_v_sb[:], in_=w_kvu_v_d[:, :])

            # ---- xT / x8T on the SP queue, piece-major --------------------
            xT = big.tile([128, ND, S], BF16, name="xT")
            x8T = big.tile([64, ND, 2, S], FP8, name="x8T")
            for p in range(NP):
                nc.sync.dma_start(
                    out=xT[:, :, 512 * p:512 * p + 512],
                    in_=x_d[:, :, 512 * p:512 * p + 512])
                nc.sync.dma_start(
                    out=x8T[:, :, :, 512 * p:512 * p + 512],
                    in_=x8_d[:, :, :, 512 * p:512 * p + 512])

            # ---- persistent tensors ---------------------------------------
            # KT8/QT8: [128p, g, plane, S]; partition 32a+p, plane pl
            # holds head 4g+a, dim 32*pl+p (fp8 for DoubleRow QK).
            KT8 = big.tile([128, 2, 2, S], FP8, name="KT8")
            QT8 = big.tile([128, 2, 2, S], FP8, name="QT8")
            # V: [128tok, chunk, head, 65] (64 vals + ones col)
            V = big.tile([128, NT, NHL, 65], BF16, name="V")
            nc.gpsimd.memset(V[:, :, :, 64:65], 1.0)
            # ctxT: [128 (2 heads x 64 dims), chunk h//2, S]
            ctxT = big.tile([128, 4, S], BF16, name="ctxT")

            # ---- work-unit emitters (pool + evac engine parameterized) ----
            def evac(on_act, dst, src, bias):
                if on_act:
                    nc.scalar.activation(dst, src, AF.Identity, bias=bias)
                else:
                    nc.vector.tensor_scalar_add(dst, src, bias)

            def unit_kv(pool, p):
                off = 512 * p
                kvp = pool.tile([128, 512], F32, tag="wk")
                for dc in range(ND):
                    nc.tensor.matmul(
                        kvp[:], w_kvc_sb[:, dc, :],
                        xT[:, dc, off:off + 512],
                        start=(dc == 0), stop=(dc == ND - 1))
                kvs = latp.tile([128, 512], BF16, tag="kvs")
                nc.vector.tensor_scalar_add(kvs[:], kvp[:], b_kvc_sb[:, 0:1])
                return kvs

            def unit_q(pool, p, qh, q8):
                off = 512 * p
                qp = pool.tile([128, 512], F32, tag="wk")
                for o in (0, 256):
                    for dc in range(ND):
                        nc.tensor.matmul(
                            qp[:, o:o + 256],
                            w_qc8_sb[:, dc, :, 128 * qh:128 * qh + 128],
                            x8T[:, dc, :, off + o:off + o + 256],
                            start=(dc == 0), stop=(dc == ND - 1),
                            perf_mode=DR)
                nc.vector.tensor_scalar_add(q8[:, qh, :], qp[:],
                                            b_qc_sb[:, qh:qh + 1])

            def unit_KT(pool, p, j, kvs, on_act=False):
                off = 512 * p
                kp = pool.tile([128, 512], F32, tag="wk")
                nc.tensor.matmul(kp[:], w_kvu_k_sb[:, 128 * j:128 * j + 128],
                                 kvs[:], start=True, stop=True)
                evac(on_act, KT8[:, j // 2, j % 2, off:off + 512], kp[:],
                     b_kvu_k_sb[:, j:j + 1])

            def unit_QT(pool, p, j, q8, on_act=False):
                off = 512 * p
                qp = pool.tile([128, 512], F32, tag="wk")
                for o in (0, 256):
                    nc.tensor.matmul(
                        qp[:, o:o + 256], w_qu8_sb[:, :, 128 * j:128 * j + 128],
                        q8[:, :, o:o + 256],
                        start=True, stop=True, perf_mode=DR)
                evac(on_act, QT8[:, j // 2, j % 2, off:off + 512], qp[:],
                     b_qu_sb[:, j:j + 1])

            def unit_V(pool, p, q, kvs):
                k = 4 * p + q
                vp = pool.tile([128, 512], F32, tag="wk")
                nc.tensor.matmul(vp[:], kvs[:, 128 * q:128 * q + 128],
                                 w_kvu_v_sb[:], start=True, stop=True)
                nc.vector.tensor_tensor(
                    V[:, k, :, 0:64],
                    vp[:].rearrange("p (h c) -> p h c", c=64),
                    bvb[:].rearrange("p (h c) -> p h c", c=64), ALU.add)

            def piece_units(pool, p, on_act=False):
                state = {}

                def mk_kv():
                    state["kvs"] = unit_kv(pool, p)

                def mk_q8():
                    q8 = latp.tile([128, 2, 512], FP8, tag="q8")
                    state["q8"] = q8
                    unit_q(pool, p, 0, q8)
                yield mk_kv
                yield mk_q8
                yield lambda: unit_q(pool, p, 1, state["q8"])
                for j in range(4):
                    yield lambda j=j: unit_KT(pool, p, j, state["kvs"], on_act)
                for j in range(4):
                    yield lambda j=j: unit_QT(pool, p, j, state["q8"], on_act)
                for q in range(4):
                    yield lambda q=q: unit_V(pool, p, q, state["kvs"])

            def unit_E(pool, si, o, on_act=False):
                op = pool.tile([128, 512], F32, tag="wk")
                for cc in range(4):
                    nc.tensor.matmul(
                        op[:], ctxT[:, cc, 128 * si:128 * si + 128],
                        w_o_sb[:, cc, 512 * o:512 * o + 512],
                        start=(cc == 0), stop=(cc == 3))
                ob = obp.tile([128, 512], F32, tag="ob")
                if on_act:
                    nc.scalar.activation(ob[:], op[:], AF.Identity, bias=0.0)
                else:
                    nc.vector.tensor_copy(ob[:], op[:])
                nc.sync.dma_start(
                    out=out_d[128 * si:128 * si + 128, 512 * o:512 * o + 512],
                    in_=ob[:])

            class Filler:
                """Dispenses queued work units evenly over `slots` calls."""
                def __init__(self, units, slots):
                    self.units = list(units)
                    self.slots = max(1, slots)
                    self.acc = 0.0
                    self.rate = len(self.units) / self.slots

                def __call__(self):
                    self.acc += self.rate
                    while self.acc >= 1.0 and self.units:
                        self.units.pop(0)()
                        self.acc -= 1.0

                def drain(self):
                    while self.units:
                        self.units.pop(0)()

            # ---- pieces 0..NP/2-1: deep scoped PSUM pool, ACT evacs -------
            with tc.tile_pool(name="pwk0", bufs=4, space="PSUM") as pwk0:
                for p in range(NP // 2):
                    for u in piece_units(pwk0, p, on_act=True):
                        u()

            # deferred heavy DMAs (needed only from phase E onwards)
            w_o_sb = wts.tile([128, 4, DIM], BF16, name="w_o_sb")
            nc.scalar.dma_start(out=w_o_sb[:], in_=w_o_d[:, :, :])

            def attn_head(j, h, filler, psc, pctx):
                """Attention for s-half j, local head h; pulls filler work
                after each key chunk to keep PE fed while ACT runs exp."""
                s0 = SH * j
                kmax = (SH // 128) * (j + 1)
                nbank = SH // 512
                last_k = {
                    bi: min(kmax - 1, (s0 + 512 * (bi + 1)) // 128 - 1)
                    for bi in range(nbank)
                }
                g, a = h // 4, h % 4
                ctx = pctx.tile([65, SH], F32, tag="ctx")
                for k in range(kmax):
                    t0 = 128 * k
                    ss = max(s0, t0)
                    fd = s0 + SH - ss
                    rel = ss - s0
                    sc = psc.tile([128, SH], F32, tag="sc")
                    for o2, w2 in _pieces(fd, 256):
                        nc.tensor.matmul(
                            sc[:, o2:o2 + w2],
                            KT8[32 * a:32 * a + 32, g, :, t0:t0 + 128],
                            QT8[32 * a:32 * a + 32, g, :, ss + o2:ss + o2 + w2],
                            start=True, stop=True, perf_mode=DR,
                            tile_position=(32 * a, 0))
                    ex = exb.tile([128, SH], BF16, tag="ex")
                    nc.scalar.activation(ex[:, :fd], sc[:, :fd],
                                         AF.Exp, scale=0.125)
                    if t0 >= s0:
                        nc.gpsimd.affine_select(
                            out=ex[:, 0:128], in_=ex[:, 0:128],
                            pattern=[[1, 128]],
                            compare_op=ALU.is_ge,
                            fill=0.0, base=0, channel_multiplier=-1)
                    for bi in range(nbank):
                        a2 = max(rel, 512 * bi)
                        b2 = min(SH, 512 * bi + 512)
                        if a2 >= b2:
                            continue
                        nc.tensor.matmul(
                            ctx[:, a2:b2], V[:, k, h, :],
                            ex[:, a2 - rel:b2 - rel],
                            start=(k == 0), stop=(k == last_k[bi]))
                    filler()
                # normalize: ctx[0:64] * (1/ctx[64]) -> ctxT slice
                rec = nrm.tile([1, SH], F32, tag="rec")
                nc.vector.reciprocal(rec[:], ctx[64:65, :])
                rbc = nrm.tile([64, SH], F32, tag="rbc")
                nc.gpsimd.partition_broadcast(rbc[:], rec[0:1, :])
                po = 64 * (h % 2)
                nc.vector.tensor_tensor(
                    ctxT[po:po + 64, h // 2, s0:s0 + SH],
                    ctx[0:64, :], rbc[:], ALU.mult)

            # ---- attention (+ pieces 2-3 and first-half out-proj fillers) -
            with (
                tc.tile_pool(name="psc", bufs=2, space="PSUM") as psc,
                tc.tile_pool(name="pctx", bufs=1, space="PSUM") as pctx,
                tc.tile_pool(name="pwk", bufs=2, space="PSUM") as pwk,
            ):
                units_j0 = [u for p in range(NP // 2, NP)
                            for u in piece_units(pwk, p)]
                f0 = Filler(units_j0, NHL * (SH // 128))
                for h in range(NHL):
                    attn_head(0, h, f0, psc, pctx)
                f0.drain()

                units_j1 = [
                    (lambda si=si, o=o: unit_E(pwk, si, o))
                    for si in range(NT // 2) for o in range(2)
                ]
                f1 = Filler(units_j1, NHL * (SH // 128) * 2)
                for h in range(NHL):
                    attn_head(1, h, f1, psc, pctx)
                f1.drain()

            # ---- tail: remaining out-proj with a deep pool, ACT evacs -----
            with tc.tile_pool(name="ptl", bufs=4, space="PSUM") as ptl:
                for si in range(NT // 2, NT):
                    for o in range(2):
                        unit_E(ptl, si, o, on_act=True)

    nc.finalize()
    return nc


def _perm512():
    """Column permutation for w_kvu_k / w_qu so that PSUM chunk j, row
    32a+p corresponds to head 4*(j//2)+a, dim 32*(j%2)+p."""
    perm = np.empty(512, dtype=np.int64)
    for j in range(4):
        for a in range(4):
            for p in range(32):
                perm[128 * j + 32 * a + p] = 64 * (4 * (j // 2) + a) + 32 * (j % 2) + p
    return perm


def shard_inputs(inputs, S=2048):
    """Build the 8 per-core input maps from full inputs (host-side prep)."""
    f32 = lambda a: np.ascontiguousarray(np.asarray(a, dtype=np.float32))
    bf = lambda a: np.ascontiguousarray(
        np.asarray(a, dtype=np.float32).astype(ml_dtypes.bfloat16))
    fp8 = lambda a: np.ascontiguousarray(
        np.asarray(a, dtype=np.float32).astype(ml_dtypes.float8_e4m3))
    x = f32(inputs["x"])
    w_kvc, b_kvc = f32(inputs["w_kvc"]), f32(inputs["b_kvc"])
    w_kvu, b_kvu = f32(inputs["w_kvu"]), f32(inputs["b_kvu"])
    w_qc, b_qc = f32(inputs["w_qc"]), f32(inputs["b_qc"])
    w_qu, b_qu = f32(inputs["w_qu"]), f32(inputs["b_qu"])
    w_o, b_o = f32(inputs["w_o"]), f32(inputs["b_o"])
    perm = _perm512()
    in_maps = []
    for core in range(NCORES):
        b = core // 2
        g2 = core % 2
        ks = slice(512 * g2, 512 * g2 + 512)            # K-feature slice
        vs = slice(DIM + 512 * g2, DIM + 512 * g2 + 512)  # V-feature slice
        in_maps.append({
            "x": bf(x[b].T.reshape(ND, 128, S).transpose(1, 0, 2)),
            "x8": fp8(x[b].T.reshape(ND, 2, 64, S).transpose(2, 0, 1, 3)),
            "w_kvc": bf(w_kvc.reshape(ND, 128, LAT).transpose(1, 0, 2)),
            "w_qc8": fp8(w_qc.reshape(ND, 2, 64, QR).transpose(2, 0, 1, 3)),
            "w_kvu_k": bf(w_kvu[:, ks][:, perm]),
            "w_qu8": fp8(w_qu[:, ks][:, perm].reshape(2, 128, 512).transpose(1, 0, 2)),
            "w_kvu_v": bf(w_kvu[:, vs]),
            "w_o": bf(w_o[ks, :].reshape(4, 128, DIM).transpose(1, 0, 2)),
            "b_kvc": f32(b_kvc.reshape(LAT, 1)),
            "b_qc": f32(b_qc.reshape(2, 128).T),
            "b_qu": f32(b_qu[ks][perm].reshape(4, 128).T),
            "b_kvu_k": f32(b_kvu[ks][perm].reshape(4, 128).T),
            "b_kvu_v": f32(b_kvu[vs].reshape(1, 512)),
        })
    return in_maps


def kernel(**inputs) -> np.ndarray:
    from concourse.bass_utils import run_bass_kernel_spmd

    x = np.asarray(inputs["x"])
    S = x.shape[1]
    nc = build_mla(S=S)
    in_maps = shard_inputs(inputs, S=S)
    res = run_bass_kernel_spmd(nc, in_maps, list(range(NCORES))).results
    b_o = np.asarray(inputs["b_o"], dtype=np.float32)
    out = np.empty((B, S, DIM), dtype=np.float32)
    for b in range(B):
        out[b] = res[2 * b]["out"] + res[2 * b + 1]["out"] + b_o
    return out


# revision 12
# speedup vs baseline: 1.4206x; 1.0139x over previous
"""MLA (multi-head latent attention) Bass kernel for Trainium2, 8 NeuronCores.

Sharding: core i handles batch b = i // 2 and head-group g2 = i % 2
(8 of the 16 heads).  Each core computes a partial output
(its heads' contribution through out_proj); the host sums the two
partials per batch and adds b_o.

Design (ACT-bound; softmax exp on ScalarE is the per-core floor):
  - Host pre-lays-out everything: x transposed to bf16 xT [128,8,S] and
    fp8 x8T [64,8,2,S]; weights pre-cast (bf16 / fp8), K/Q up-projection
    columns pre-permuted so the fp8 DoubleRow layout falls out of plain
    PSUM evacuations.
  - QK^T runs in fp8e4 DoubleRow: KT8/QT8 stored [128p, g, plane, S]
    (partition 32a+p, plane pl = head 4g+a, dim 32pl+p); one matmul
    contracts all 64 head dims at 0.5 cycles/col.  The whole Q path
    (x->q_lat->QT) is fp8 DoubleRow too - it only feeds softmax scores,
    which tolerate fp8 noise.  V/out paths stay bf16.
  - Emission order software-pipelines: pieces 0-1 up front (deep scoped
    PSUM pool, KT/QT evacuations on the then-idle ACT engine), pieces
    2-3 as fillers inside j=0 attention, out-proj of the first token
    half as fillers inside j=1, remainder in a deep-pool tail with ACT
    evacuations.
  - PSUM: attention = scores [128,1024]x2bufs (4 banks) + ctx [65,1024]
    (2) + filler work tiles [128,512]x2 (2).
"""

import numpy as np
import ml_dtypes

import concourse.bass as bass
import concourse.bacc as bacc
import concourse.mybir as mybir
import concourse.tile as tile

DIM = 1024
NUM_HEADS = 16
HEAD_DIM = 64
LAT = 128
QR = 256
B = 4
NCORES = 8
ND = DIM // 128       # 8 d-chunks
NHL = 8               # heads per core
F32 = mybir.dt.float32
BF16 = mybir.dt.bfloat16
FP8 = mybir.dt.float8e4
AF = mybir.ActivationFunctionType
ALU = mybir.AluOpType
DR = mybir.MatmulPerfMode.DoubleRow


def _pieces(total, w=512):
    return [(o, min(w, total - o)) for o in range(0, total, w)]


def build_mla(S=2048):
    """Build the per-core Bass program (same SPMD program on all 8 cores)."""
    assert S % 1024 == 0
    SH = S // 2           # s-half width
    NT = S // 128         # number of 128-token chunks
    NP = S // 512         # number of 512-token pieces

    nc = bacc.Bacc()

    x_d = nc.declare_dram_parameter("x", [128, ND, S], BF16, isOutput=False)
    x8_d = nc.declare_dram_parameter("x8", [64, ND, 2, S], FP8, isOutput=False)
    w_kvc_d = nc.declare_dram_parameter("w_kvc", [128, ND, LAT], BF16, isOutput=False)
    w_qc8_d = nc.declare_dram_parameter("w_qc8", [64, ND, 2, QR], FP8, isOutput=False)
    w_kvu_k_d = nc.declare_dram_parameter("w_kvu_k", [128, 512], BF16, isOutput=False)
    w_qu8_d = nc.declare_dram_parameter("w_qu8", [128, 2, 512], FP8, isOutput=False)
    w_kvu_v_d = nc.declare_dram_parameter("w_kvu_v", [128, 512], BF16, isOutput=False)
    w_o_d = nc.declare_dram_parameter("w_o", [128, 4, DIM], BF16, isOutput=False)
    b_kvc_d = nc.declare_dram_parameter("b_kvc", [LAT, 1], F32, isOutput=False)
    b_qc_d = nc.declare_dram_parameter("b_qc", [128, 2], F32, isOutput=False)
    b_qu_d = nc.declare_dram_parameter("b_qu", [128, 4], F32, isOutput=False)
    b_kvu_k_d = nc.declare_dram_parameter("b_kvu_k", [128, 4], F32, isOutput=False)
    b_kvu_v_d = nc.declare_dram_parameter("b_kvu_v", [1, 512], F32, isOutput=False)
    out_d = nc.declare_dram_parameter("out", [S, DIM], F32, isOutput=True)

    with tile.TileContext(nc) as tc:
        with (
            tc.tile_pool(name="wts", bufs=1) as wts,
            tc.tile_pool(name="big", bufs=1) as big,
            tc.tile_pool(name="lat", bufs=2) as latp,
            tc.tile_pool(name="exb", bufs=4) as exb,
            tc.tile_pool(name="nrm", bufs=2) as nrm,
            tc.tile_pool(name="obp", bufs=4) as obp,
        ):
            # ---- early ACT-queue DMAs (small biases + proj weights) -------
            b_kvc_sb = wts.tile([128, 1], F32, name="b_kvc_sb")
            nc.scalar.dma_start(out=b_kvc_sb[:], in_=b_kvc_d[:, :])
            b_qc_sb = wts.tile([128, 2], F32, name="b_qc_sb")
            nc.scalar.dma_start(out=b_qc_sb[:], in_=b_qc_d[:, :])
            b_qu_sb = wts.tile([128, 4], F32, name="b_qu_sb")
            nc.scalar.dma_start(out=b_qu_sb[:], in_=b_qu_d[:, :])
            b_kvu_k_sb = wts.tile([128, 4], F32, name="b_kvu_k_sb")
            nc.scalar.dma_start(out=b_kvu_k_sb[:], in_=b_kvu_k_d[:, :])
            bv_row = wts.tile([1, 512], F32, name="bv_row")
            nc.scalar.dma_start(out=bv_row[:], in_=b_kvu_v_d[:, :])
            bvb = wts.tile([128, 512], F32, name="bvb")
            nc.gpsimd.partition_broadcast(bvb[:], bv_row[0:1, :])
            w_kvc_sb = wts.tile([128, ND, LAT], BF16, name="w_kvc_sb")
            nc.scalar.dma_start(out=w_kvc_sb[:], in_=w_kvc_d[:, :, :])
            w_qc8_sb = wts.tile([64, ND, 2, QR], FP8, name="w_qc8_sb")
            nc.scalar.dma_start(out=w_qc8_sb[:], in_=w_qc8_d[:, :, :, :])
            w_qu8_sb = wts.tile([128, 2, 512], FP8, name="w_qu8_sb")
            nc.scalar.dma_start(out=w_qu8_sb[:], in_=w_qu8_d[:, :, :])
            w_kvu_k_sb = wts.tile([128, 512], BF16, name="w_kvu_k_sb")
            nc.scalar.dma_start(out=w_kvu_k_sb[:], in_=w_kvu_k_d[:, :])
            w_kvu_v_sb = wts.tile([128, 512], BF16, name="w_kvu_v_sb")
            nc.scalar.dma_start(out=w_kvu_v_sb[:], in_=w_kvu_v_d[:, :])

            # ---- xT / x8T on the SP queue, piece-major --------------------
            xT = big.tile([128, ND, S], BF16, name="xT")
            x8T = big.tile([64, ND, 2, S], FP8, name="x8T")
            for p in range(NP):
                nc.sync.dma_start(
                    out=xT[:, :, 512 * p:512 * p + 512],
                    in_=x_d[:, :, 512 * p:512 * p + 512])
                nc.sync.dma_start(
                    out=x8T[:, :, :, 512 * p:512 * p + 512],
                    in_=x8_d[:, :, :, 512 * p:512 * p + 512])

            # w_o rides the SP queue after xT/x8 (needed only in phase E)
            w_o_sb = wts.tile([128, 4, DIM], BF16, name="w_o_sb")
            nc.sync.dma_start(out=w_o_sb[:], in_=w_o_d[:, :, :])

            # ---- persistent tensors ---------------------------------------
            # KT8/QT8: [128p, g, plane, S]; partition 32a+p, plane pl
            # holds head 4g+a, dim 32*pl+p (fp8 for DoubleRow QK).
            KT8 = big.tile([128, 2, 2, S], FP8, name="KT8")
            QT8 = big.tile([128, 2, 2, S], FP8, name="QT8")
            # V: [128tok, chunk, head, 65] (64 vals + ones col)
            V = big.tile([128, NT, NHL, 65], BF16, name="V")
            nc.gpsimd.memset(V[:, :, :, 64:65], 1.0)
            # ctxT: [128 (2 heads x 64 dims), chunk h//2, S]
            ctxT = big.tile([128, 4, S], BF16, name="ctxT")

            # ---- work-unit emitters (pool + evac engine parameterized) ----
            def evac(on_act, dst, src, bias):
                if on_act:
                    nc.scalar.activation(dst, src, AF.Identity, bias=bias)
                else:
                    nc.vector.tensor_scalar_add(dst, src, bias)

            def unit_kv(pool, p):
                off = 512 * p
                kvp = pool.tile([128, 512], F32, tag="wk")
                for dc in range(ND):
                    nc.tensor.matmul(
                        kvp[:], w_kvc_sb[:, dc, :],
                        xT[:, dc, off:off + 512],
                        start=(dc == 0), stop=(dc == ND - 1))
                kvs = latp.tile([128, 512], BF16, tag="kvs")
                nc.vector.tensor_scalar_add(kvs[:], kvp[:], b_kvc_sb[:, 0:1])
                return kvs

            def unit_q(pool, p, qh, q8):
                off = 512 * p
                qp = pool.tile([128, 512], F32, tag="wk")
                for o in (0, 256):
                    for dc in range(ND):
                        nc.tensor.matmul(
                            qp[:, o:o + 256],
                            w_qc8_sb[:, dc, :, 128 * qh:128 * qh + 128],
                            x8T[:, dc, :, off + o:off + o + 256],
                            start=(dc == 0), stop=(dc == ND - 1),
                            perf_mode=DR)
                nc.vector.tensor_scalar_add(q8[:, qh, :], qp[:],
                                            b_qc_sb[:, qh:qh + 1])

            def unit_KT(pool, p, j, kvs, on_act=False):
                off = 512 * p
                kp = pool.tile([128, 512], F32, tag="wk")
                nc.tensor.matmul(kp[:], w_kvu_k_sb[:, 128 * j:128 * j + 128],
                                 kvs[:], start=True, stop=True)
                evac(on_act, KT8[:, j // 2, j % 2, off:off + 512], kp[:],
                     b_kvu_k_sb[:, j:j + 1])

            def unit_QT(pool, p, j, q8, on_act=False):
                off = 512 * p
                qp = pool.tile([128, 512], F32, tag="wk")
                for o in (0, 256):
                    nc.tensor.matmul(
                        qp[:, o:o + 256], w_qu8_sb[:, :, 128 * j:128 * j + 128],
                        q8[:, :, o:o + 256],
                        start=True, stop=True, perf_mode=DR)
                evac(on_act, QT8[:, j // 2, j % 2, off:off + 512], qp[:],
                     b_qu_sb[:, j:j + 1])

            def unit_V(pool, p, q, kvs):
                k = 4 * p + q
                vp = pool.tile([128, 512], F32, tag="wk")
                nc.tensor.matmul(vp[:], kvs[:, 128 * q:128 * q + 128],
                                 w_kvu_v_sb[:], start=True, stop=True)
                nc.vector.tensor_tensor(
                    V[:, k, :, 0:64],
                    vp[:].rearrange("p (h c) -> p h c", c=64),
                    bvb[:].rearrange("p (h c) -> p h c", c=64), ALU.add)

            def piece_units(pool, p, on_act=False):
                state = {}

                def mk_kv():
                    state["kvs"] = unit_kv(pool, p)

                def mk_q8():
                    q8 = latp.tile([128, 2, 512], FP8, tag="q8")
                    state["q8"] = q8
                    unit_q(pool, p, 0, q8)
                yield mk_kv
                yield mk_q8
                yield lambda: unit_q(pool, p, 1, state["q8"])
                for j in range(4):
                    yield lambda j=j: unit_KT(pool, p, j, state["kvs"])
                for j in range(4):
                    yield lambda j=j: unit_QT(pool, p, j, state["q8"], on_act)
                for q in range(4):
                    yield lambda q=q: unit_V(pool, p, q, state["kvs"])

            def unit_E(pool, si, o, on_act=False):
                op = pool.tile([128, 512], F32, tag="wk")
                for cc in range(4):
                    nc.tensor.matmul(
                        op[:], ctxT[:, cc, 128 * si:128 * si + 128],
                        w_o_sb[:, cc, 512 * o:512 * o + 512],
                        start=(cc == 0), stop=(cc == 3))
                ob = obp.tile([128, 512], F32, tag="ob")
                if on_act:
                    nc.scalar.activation(ob[:], op[:], AF.Identity, bias=0.0)
                else:
                    nc.vector.tensor_copy(ob[:], op[:])
                nc.sync.dma_start(
                    out=out_d[128 * si:128 * si + 128, 512 * o:512 * o + 512],
                    in_=ob[:])

            class Filler:
                """Dispenses queued work units evenly over `slots` calls."""
                def __init__(self, units, slots):
                    self.units = list(units)
                    self.slots = max(1, slots)
                    self.acc = 0.0
                    self.rate = len(self.units) / self.slots

                def __call__(self):
                    self.acc += self.rate
                    while self.acc >= 1.0 and self.units:
                        self.units.pop(0)()
                        self.acc -= 1.0

                def drain(self):
                    while self.units:
                        self.units.pop(0)()

            # ---- pieces 0..NP/2-1: deep scoped PSUM pool, ACT evacs -------
            with tc.tile_pool(name="pwk0", bufs=4, space="PSUM") as pwk0:
                for p in range(NP // 2):
                    for u in piece_units(pwk0, p, on_act=True):
                        u()


            def attn_phase(j, heads, filler, psc, pctx):
                """Attention for s-half j over `heads`, emitted with QK one
                chunk ahead of PV so exp never waits at head boundaries."""
                s0 = SH * j
                kmax = (SH // 128) * (j + 1)
                nbank = SH // 512
                last_k = {
                    bi: min(kmax - 1, (s0 + 512 * (bi + 1)) // 128 - 1)
                    for bi in range(nbank)
                }
                recs = []
                for h in heads:
                    g, a = h // 4, h % 4
                    hst = {}
                    for k in range(kmax):
                        t0 = 128 * k
                        ss = max(s0, t0)
                        fd = s0 + SH - ss
                        rel = ss - s0
                        rec = {}

                        def qk(rec=rec, g=g, a=a, t0=t0, ss=ss, fd=fd):
                            sc = psc.tile([128, SH], F32, tag="sc")
                            rec["sc"] = sc  # noqa
                            for o2, w2 in _pieces(fd, 256):
                                nc.tensor.matmul(
                                    sc[:, o2:o2 + w2],
                                    KT8[32 * a:32 * a + 32, g, :, t0:t0 + 128],
                                    QT8[32 * a:32 * a + 32, g, :,
                                        ss + o2:ss + o2 + w2],
                                    start=True, stop=True, perf_mode=DR,
                                    tile_position=(32 * a, 0))

                        def expaff(rec=rec, fd=fd, diag=(t0 >= s0)):
                            ex = exb.tile([128, SH], BF16, tag="ex")
                            rec["ex"] = ex  # noqa
                            nc.scalar.activation(ex[:, :fd], rec["sc"][:, :fd],
                                                 AF.Exp, scale=0.125)
                            if diag:
                                nc.gpsimd.affine_select(
                                    out=ex[:, 0:128], in_=ex[:, 0:128],
                                    pattern=[[1, 128]],
                                    compare_op=ALU.is_ge,
                                    fill=0.0, base=0, channel_multiplier=-1)

                        def pv(rec=rec, hst=hst, h=h, k=k, rel=rel):
                            if k == 0:
                                ctx = pctx.tile([65, SH], F32, tag="ctx")
                                hst["ctx"] = ctx
                            for bi in range(nbank):
                                a2 = max(rel, 512 * bi)
                                b2 = min(SH, 512 * bi + 512)
                                if a2 >= b2:
                                    continue
                                nc.tensor.matmul(
                                    hst["ctx"][:, a2:b2], V[:, k, h, :],
                                    rec["ex"][:, a2 - rel:b2 - rel],
                                    start=(k == 0), stop=(k == last_k[bi]))

                        rec.update(qk=qk, expaff=expaff, pv=pv)
                        if k == kmax - 1:
                            def norm(hst=hst, h=h):
                                ctx = hst["ctx"]
                                rc = nrm.tile([1, SH], F32, tag="rec")
                                nc.vector.reciprocal(rc[:], ctx[64:65, :])
                                rbc = nrm.tile([64, SH], F32, tag="rbc")
                                nc.gpsimd.partition_broadcast(rbc[:], rc[0:1, :])
                                po = 64 * (h % 2)
                                nc.vector.tensor_tensor(
                                    ctxT[po:po + 64, h // 2, s0:s0 + SH],
                                    ctx[0:64, :], rbc[:], ALU.mult)
                            rec["norm"] = norm
                        recs.append(rec)
                recs[0]["qk"]()
                for i, rec in enumerate(recs):
                    rec["expaff"]()
                    if i + 1 < len(recs):
                        recs[i + 1]["qk"]()
                    rec["pv"]()
                    if "norm" in rec:
                        rec["norm"]()
                    filler()

            # ---- attention (+ pieces 2-3 and first-half out-proj fillers) -
            with (
                tc.tile_pool(name="psc", bufs=2, space="PSUM") as psc,
                tc.tile_pool(name="pctx", bufs=1, space="PSUM") as pctx,
                tc.tile_pool(name="pwk", bufs=2, space="PSUM") as pwk,
            ):
                units_j0 = [u for p in range(NP // 2, NP)
                            for u in piece_units(pwk, p)]
                f0 = Filler(units_j0, NHL * (SH // 128))
                attn_phase(0, range(NHL), f0, psc, pctx)
                f0.drain()

                units_j1 = [
                    (lambda si=si, o=o: unit_E(pwk, si, o))
                    for si in range(NT // 2) for o in range(2)
                ]
                f1 = Filler(units_j1, NHL * (SH // 128) * 2)
                attn_phase(1, range(NHL), f1, psc, pctx)
                f1.drain()

            # ---- tail: remaining out-proj with a deep pool, ACT evacs -----
            with tc.tile_pool(name="ptl", bufs=4, space="PSUM") as ptl:
                for si in range(NT // 2, NT):
                    for o in range(2):
                        unit_E(ptl, si, o, on_act=True)

    nc.finalize()
    return nc


def _perm512():
    """Column permutation for w_kvu_k / w_qu so that PSUM chunk j, row
    32a+p corresponds to head 4*(j//2)+a, dim 32*(j%2)+p."""
    perm = np.empty(512, dtype=np.int64)
    for j in range(4):
        for a in range(4):
            for p in range(32):
                perm[128 * j + 32 * a + p] = 64 * (4 * (j // 2) + a) + 32 * (j % 2) + p
    return perm


def shard_inputs(inputs, S=2048):
    """Build the 8 per-core input maps from full inputs (host-side prep)."""
    f32 = lambda a: np.ascontiguousarray(np.asarray(a, dtype=np.float32))
    bf = lambda a: np.ascontiguousarray(
        np.asarray(a, dtype=np.float32).astype(ml_dtypes.bfloat16))
    fp8 = lambda a: np.ascontiguousarray(
        np.asarray(a, dtype=np.float32).astype(ml_dtypes.float8_e4m3))
    x = f32(inputs["x"])
    w_kvc, b_kvc = f32(inputs["w_kvc"]), f32(inputs["b_kvc"])
    w_kvu, b_kvu = f32(inputs["w_kvu"]), f32(inputs["b_kvu"])
    w_qc, b_qc = f32(inputs["w_qc"]), f32(inputs["b_qc"])
    w_qu, b_qu = f32(inputs["w_qu"]), f32(inputs["b_qu"])
    w_o, b_o = f32(inputs["w_o"]), f32(inputs["b_o"])
    perm = _perm512()
    in_maps = []
    for core in range(NCORES):
        b = core // 2
        g2 = core % 2
        ks = slice(512 * g2, 512 * g2 + 512)            # K-feature slice
        vs = slice(DIM + 512 * g2, DIM + 512 * g2 + 512)  # V-feature slice
        in_maps.append({
            "x": bf(x[b].T.reshape(ND, 128, S).transpose(1, 0, 2)),
            "x8": fp8(x[b].T.reshape(ND, 2, 64, S).transpose(2, 0, 1, 3)),
            "w_kvc": bf(w_kvc.reshape(ND, 128, LAT).transpose(1, 0, 2)),
            "w_qc8": fp8(w_qc.reshape(ND, 2, 64, QR).transpose(2, 0, 1, 3)),
            "w_kvu_k": bf(w_kvu[:, ks][:, perm]),
            "w_qu8": fp8(w_qu[:, ks][:, perm].reshape(2, 128, 512).transpose(1, 0, 2)),
            "w_kvu_v": bf(w_kvu[:, vs]),
            "w_o": bf(w_o[ks, :].reshape(4, 128, DIM).transpose(1, 0, 2)),
            "b_kvc": f32(b_kvc.reshape(LAT, 1)),
            "b_qc": f32(b_qc.reshape(2, 128).T),
            "b_qu": f32(b_qu[ks][perm].reshape(4, 128).T),
            "b_kvu_k": f32(b_kvu[ks][perm].reshape(4, 128).T),
            "b_kvu_v": f32(b_kvu[vs].reshape(1, 512)),
        })
    return in_maps


def kernel(**inputs) -> np.ndarray:
    from concourse.bass_utils import run_bass_kernel_spmd

    x = np.asarray(inputs["x"])
    S = x.shape[1]
    nc = build_mla(S=S)
    in_maps = shard_inputs(inputs, S=S)
    res = run_bass_kernel_spmd(nc, in_maps, list(range(NCORES))).results
    b_o = np.asarray(inputs["b_o"], dtype=np.float32)
    out = np.empty((B, S, DIM), dtype=np.float32)
    for b in range(B):
        out[b] = res[2 * b]["out"] + res[2 * b + 1]["out"] + b_o
    return out


# revision 14
# speedup vs baseline: 1.4395x; 1.0133x over previous
"""MLA (multi-head latent attention) Bass kernel for Trainium2, 8 NeuronCores.

Sharding: core i handles batch b = i // 2 and head-group g2 = i % 2
(8 of the 16 heads).  Each core computes a partial output
(its heads' contribution through out_proj); the host sums the two
partials per batch and adds b_o.

Design (ACT-bound; softmax exp on ScalarE is the per-core floor):
  - Host pre-lays-out everything: x transposed to bf16 xT [128,8,S] and
    fp8 x8T [64,8,2,S]; weights pre-cast (bf16 / fp8), K/Q up-projection
    columns pre-permuted so the fp8 DoubleRow layout falls out of plain
    PSUM evacuations.
  - QK^T runs in fp8e4 DoubleRow: KT8/QT8 stored [128p, g, plane, S]
    (partition 32a+p, plane pl = head 4g+a, dim 32pl+p); one matmul
    contracts all 64 head dims at 0.5 cycles/col.  The whole Q path
    (x->q_lat->QT) is fp8 DoubleRow too - it only feeds softmax scores,
    which tolerate fp8 noise.  V/out paths stay bf16.
  - Emission order software-pipelines: pieces 0-1 up front (deep scoped
    PSUM pool, KT/QT evacuations on the then-idle ACT engine), pieces
    2-3 as fillers inside j=0 attention, out-proj of the first token
    half as fillers inside j=1, remainder in a deep-pool tail with ACT
    evacuations.
  - PSUM: attention = scores [128,1024]x2bufs (4 banks) + ctx [65,1024]
    (2) + filler work tiles [128,512]x2 (2).
"""

import numpy as np
import ml_dtypes

import concourse.bass as bass
import concourse.bacc as bacc
import concourse.mybir as mybir
import concourse.tile as tile

DIM = 1024
NUM_HEADS = 16
HEAD_DIM = 64
LAT = 128
QR = 256
B = 4
NCORES = 8
ND = DIM // 128       # 8 d-chunks
NHL = 8               # heads per core
F32 = mybir.dt.float32
BF16 = mybir.dt.bfloat16
FP8 = mybir.dt.float8e4
AF = mybir.ActivationFunctionType
ALU = mybir.AluOpType
DR = mybir.MatmulPerfMode.DoubleRow


def _pieces(total, w=512):
    return [(o, min(w, total - o)) for o in range(0, total, w)]


def build_mla(S=2048):
    """Build the per-core Bass program (same SPMD program on all 8 cores)."""
    assert S % 1024 == 0
    SH = S // 2           # s-half width
    NT = S // 128         # number of 128-token chunks
    NP = S // 512         # number of 512-token pieces

    nc = bacc.Bacc()

    x_d = nc.declare_dram_parameter("x", [128, ND, S], BF16, isOutput=False)
    x8_d = nc.declare_dram_parameter("x8", [64, ND, 2, S], FP8, isOutput=False)
    w_kvc_d = nc.declare_dram_parameter("w_kvc", [128, ND, LAT], BF16, isOutput=False)
    w_qc8_d = nc.declare_dram_parameter("w_qc8", [64, ND, 2, QR], FP8, isOutput=False)
    w_kvu_k_d = nc.declare_dram_parameter("w_kvu_k", [128, 512], BF16, isOutput=False)
    w_qu8_d = nc.declare_dram_parameter("w_qu8", [128, 2, 512], FP8, isOutput=False)
    w_kvu_v_d = nc.declare_dram_parameter("w_kvu_v", [128, 512], BF16, isOutput=False)
    w_o_d = nc.declare_dram_parameter("w_o", [128, 4, DIM], BF16, isOutput=False)
    b_kvc_d = nc.declare_dram_parameter("b_kvc", [LAT, 1], F32, isOutput=False)
    b_qc_d = nc.declare_dram_parameter("b_qc", [128, 2], F32, isOutput=False)
    b_qu_d = nc.declare_dram_parameter("b_qu", [128, 4], F32, isOutput=False)
    b_kvu_k_d = nc.declare_dram_parameter("b_kvu_k", [128, 4], F32, isOutput=False)
    b_kvu_v_d = nc.declare_dram_parameter("b_kvu_v", [1, 512], F32, isOutput=False)
    out_d = nc.declare_dram_parameter("out", [S, DIM], F32, isOutput=True)

    with tile.TileContext(nc) as tc:
        with (
            tc.tile_pool(name="wts", bufs=1) as wts,
            tc.tile_pool(name="big", bufs=1) as big,
            tc.tile_pool(name="lat", bufs=2) as latp,
            tc.tile_pool(name="exb", bufs=4) as exb,
            tc.tile_pool(name="nrm", bufs=2) as nrm,
            tc.tile_pool(name="obp", bufs=4) as obp,
        ):
            # ---- early ACT-queue DMAs: weights for the first matmuls
            # lead, then biases (needed only at evac time); a dummy exp
            # preloads the activation table off the critical path --------
            w_kvc_sb = wts.tile([128, ND, LAT], BF16, name="w_kvc_sb")
            nc.scalar.dma_start(out=w_kvc_sb[:], in_=w_kvc_d[:, :, :])
            atl = wts.tile([1, 1], F32, name="atl")
            nc.gpsimd.memset(atl[:], 0.0)
            nc.scalar.activation(atl[:], atl[:], AF.Exp, scale=1.0)
            w_qc8_sb = wts.tile([64, ND, 2, QR], FP8, name="w_qc8_sb")
            nc.scalar.dma_start(out=w_qc8_sb[:], in_=w_qc8_d[:, :, :, :])
            w_qu8_sb = wts.tile([128, 2, 512], FP8, name="w_qu8_sb")
            nc.scalar.dma_start(out=w_qu8_sb[:], in_=w_qu8_d[:, :, :])
            w_kvu_k_sb = wts.tile([128, 512], BF16, name="w_kvu_k_sb")
            nc.scalar.dma_start(out=w_kvu_k_sb[:], in_=w_kvu_k_d[:, :])
            b_kvc_sb = wts.tile([128, 1], F32, name="b_kvc_sb")
            nc.scalar.dma_start(out=b_kvc_sb[:], in_=b_kvc_d[:, :])
            b_qc_sb = wts.tile([128, 2], F32, name="b_qc_sb")
            nc.scalar.dma_start(out=b_qc_sb[:], in_=b_qc_d[:, :])
            b_qu_sb = wts.tile([128, 4], F32, name="b_qu_sb")
            nc.scalar.dma_start(out=b_qu_sb[:], in_=b_qu_d[:, :])
            b_kvu_k_sb = wts.tile([128, 4], F32, name="b_kvu_k_sb")
            nc.scalar.dma_start(out=b_kvu_k_sb[:], in_=b_kvu_k_d[:, :])
            bv_row = wts.tile([1, 512], F32, name="bv_row")
            nc.scalar.dma_start(out=bv_row[:], in_=b_kvu_v_d[:, :])
            bvb = wts.tile([128, 512], F32, name="bvb")
            nc.gpsimd.partition_broadcast(bvb[:], bv_row[0:1, :])
            w_kvu_v_sb = wts.tile([128, 512], BF16, name="w_kvu_v_sb")
            nc.scalar.dma_start(out=w_kvu_v_sb[:], in_=w_kvu_v_d[:, :])

            # ---- xT / x8T on the SP queue, piece-major --------------------
            xT = big.tile([128, ND, S], BF16, name="xT")
            x8T = big.tile([64, ND, 2, S], FP8, name="x8T")
            for p in range(NP):
                nc.sync.dma_start(
                    out=xT[:, :, 512 * p:512 * p + 512],
                    in_=x_d[:, :, 512 * p:512 * p + 512])
                nc.sync.dma_start(
                    out=x8T[:, :, :, 512 * p:512 * p + 512],
                    in_=x8_d[:, :, :, 512 * p:512 * p + 512])

            # w_o rides the SP queue after xT/x8 (needed only in phase E)
            w_o_sb = wts.tile([128, 4, DIM], BF16, name="w_o_sb")
            nc.sync.dma_start(out=w_o_sb[:], in_=w_o_d[:, :, :])

            # ---- persistent tensors ---------------------------------------
            # KT8/QT8: [128p, g, plane, S]; partition 32a+p, plane pl
            # holds head 4g+a, dim 32*pl+p (fp8 for DoubleRow QK).
            KT8 = big.tile([128, 2, 2, S], FP8, name="KT8")
            QT8 = big.tile([128, 2, 2, S], FP8, name="QT8")
            # V: [128tok, chunk, head, 65] (64 vals + ones col)
            V = big.tile([128, NT, NHL, 65], BF16, name="V")
            nc.gpsimd.memset(V[:, :, :, 64:65], 1.0)
            # ctxT: [128 (2 heads x 64 dims), chunk h//2, S]
            ctxT = big.tile([128, 4, S], BF16, name="ctxT")

            # ---- work-unit emitters (pool + evac engine parameterized) ----
            def evac(on_act, dst, src, bias):
                if on_act:
                    nc.scalar.activation(dst, src, AF.Identity, bias=bias)
                else:
                    nc.vector.tensor_scalar_add(dst, src, bias)

            def unit_kv(pool, p):
                off = 512 * p
                kvp = pool.tile([128, 512], F32, tag="wk")
                for dc in range(ND):
                    nc.tensor.matmul(
                        kvp[:], w_kvc_sb[:, dc, :],
                        xT[:, dc, off:off + 512],
                        start=(dc == 0), stop=(dc == ND - 1))
                kvs = latp.tile([128, 512], BF16, tag="kvs")
                nc.vector.tensor_scalar_add(kvs[:], kvp[:], b_kvc_sb[:, 0:1])
                return kvs

            def unit_q(pool, p, qh, q8):
                off = 512 * p
                qp = pool.tile([128, 512], F32, tag="wk")
                for o in (0, 256):
                    for dc in range(ND):
                        nc.tensor.matmul(
                            qp[:, o:o + 256],
                            w_qc8_sb[:, dc, :, 128 * qh:128 * qh + 128],
                            x8T[:, dc, :, off + o:off + o + 256],
                            start=(dc == 0), stop=(dc == ND - 1),
                            perf_mode=DR)
                nc.vector.tensor_scalar_add(q8[:, qh, :], qp[:],
                                            b_qc_sb[:, qh:qh + 1])

            def unit_KT(pool, p, j, kvs, on_act=False):
                off = 512 * p
                kp = pool.tile([128, 512], F32, tag="wk")
                nc.tensor.matmul(kp[:], w_kvu_k_sb[:, 128 * j:128 * j + 128],
                                 kvs[:], start=True, stop=True)
                evac(on_act, KT8[:, j // 2, j % 2, off:off + 512], kp[:],
                     b_kvu_k_sb[:, j:j + 1])

            def unit_QT(pool, p, j, q8, on_act=False):
                off = 512 * p
                qp = pool.tile([128, 512], F32, tag="wk")
                for o in (0, 256):
                    nc.tensor.matmul(
                        qp[:, o:o + 256], w_qu8_sb[:, :, 128 * j:128 * j + 128],
                        q8[:, :, o:o + 256],
                        start=True, stop=True, perf_mode=DR)
                evac(on_act, QT8[:, j // 2, j % 2, off:off + 512], qp[:],
                     b_qu_sb[:, j:j + 1])

            def unit_V(pool, p, q, kvs):
                k = 4 * p + q
                vp = pool.tile([128, 512], F32, tag="wk")
                nc.tensor.matmul(vp[:], kvs[:, 128 * q:128 * q + 128],
                                 w_kvu_v_sb[:], start=True, stop=True)
                nc.vector.tensor_tensor(
                    V[:, k, :, 0:64],
                    vp[:].rearrange("p (h c) -> p h c", c=64),
                    bvb[:].rearrange("p (h c) -> p h c", c=64), ALU.add)

            def piece_units(pool, p, on_act=False, only=None):
                state = {}

                def mk_kv():
                    state["kvs"] = unit_kv(pool, p)

                def mk_q8():
                    q8 = latp.tile([128, 2, 512], FP8, tag="q8")
                    state["q8"] = q8
                    unit_q(pool, p, 0, q8)
                units = [("kv", mk_kv), ("q", mk_q8),
                         ("q", lambda: unit_q(pool, p, 1, state["q8"]))]
                units += [("KT", (lambda j=j: unit_KT(pool, p, j, state["kvs"],
                                                      on_act)))
                          for j in range(4)]
                units += [("QT", (lambda j=j: unit_QT(pool, p, j, state["q8"],
                                                      on_act)))
                          for j in range(4)]
                units += [("V", (lambda q=q: unit_V(pool, p, q, state["kvs"])))
                          for q in range(4)]
                for kind, u in units:
                    if only is None or kind in only:
                        yield u

            def unit_E(pool, si, o, on_act=False):
                op = pool.tile([128, 512], F32, tag="wk")
                for cc in range(4):
                    nc.tensor.matmul(
                        op[:], ctxT[:, cc, 128 * si:128 * si + 128],
                        w_o_sb[:, cc, 512 * o:512 * o + 512],
                        start=(cc == 0), stop=(cc == 3))
                ob = obp.tile([128, 512], F32, tag="ob")
                if on_act:
                    nc.scalar.activation(ob[:], op[:], AF.Identity, bias=0.0)
                else:
                    nc.vector.tensor_copy(ob[:], op[:])
                nc.sync.dma_start(
                    out=out_d[128 * si:128 * si + 128, 512 * o:512 * o + 512],
                    in_=ob[:])

            class Filler:
                """Dispenses queued work units evenly over `slots` calls."""
                def __init__(self, units, slots):
                    self.units = list(units)
                    self.slots = max(1, slots)
                    self.acc = 0.0
                    self.rate = len(self.units) / self.slots

                def __call__(self):
                    self.acc += self.rate
                    while self.acc >= 1.0 and self.units:
                        self.units.pop(0)()
                        self.acc -= 1.0

                def drain(self):
                    while self.units:
                        self.units.pop(0)()

            # ---- pieces 0..NP/2-1: deep scoped PSUM pool, ACT evacs -------
            with tc.tile_pool(name="pwk0", bufs=4, space="PSUM") as pwk0:
                for p in range(NP // 2):
                    for u in piece_units(pwk0, p, on_act=True):
                        u()


            def attn_phase(j, heads, filler, psc, pctx):
                """Attention for s-half j over `heads`, emitted with QK one
                chunk ahead of PV so exp never waits at head boundaries."""
                s0 = SH * j
                kmax = (SH // 128) * (j + 1)
                nbank = SH // 512
                last_k = {
                    bi: min(kmax - 1, (s0 + 512 * (bi + 1)) // 128 - 1)
                    for bi in range(nbank)
                }
                recs = []
                for h in heads:
                    g, a = h // 4, h % 4
                    hst = {}
                    for k in range(kmax):
                        t0 = 128 * k
                        ss = max(s0, t0)
                        fd = s0 + SH - ss
                        rel = ss - s0
                        rec = {}

                        def qk(rec=rec, g=g, a=a, t0=t0, ss=ss, fd=fd):
                            sc = psc.tile([128, SH], F32, tag="sc")
                            rec["sc"] = sc  # noqa
                            for o2, w2 in _pieces(fd, 256):
                                nc.tensor.matmul(
                                    sc[:, o2:o2 + w2],
                                    KT8[32 * a:32 * a + 32, g, :, t0:t0 + 128],
                                    QT8[32 * a:32 * a + 32, g, :,
                                        ss + o2:ss + o2 + w2],
                                    start=True, stop=True, perf_mode=DR,
                                    tile_position=(32 * a, 0))

                        def expaff(rec=rec, fd=fd, diag=(t0 >= s0)):
                            ex = exb.tile([128, SH], BF16, tag="ex")
                            rec["ex"] = ex  # noqa
                            nc.scalar.activation(ex[:, :fd], rec["sc"][:, :fd],
                                                 AF.Exp, scale=0.125)
                            if diag:
                                nc.gpsimd.affine_select(
                                    out=ex[:, 0:128], in_=ex[:, 0:128],
                                    pattern=[[1, 128]],
                                    compare_op=ALU.is_ge,
                                    fill=0.0, base=0, channel_multiplier=-1)

                        def pv(rec=rec, hst=hst, h=h, k=k, rel=rel):
                            if k == 0:
                                ctx = pctx.tile([65, SH], F32, tag="ctx")
                                hst["ctx"] = ctx
                            for bi in range(nbank):
                                a2 = max(rel, 512 * bi)
                                b2 = min(SH, 512 * bi + 512)
                                if a2 >= b2:
                                    continue
                                nc.tensor.matmul(
                                    hst["ctx"][:, a2:b2], V[:, k, h, :],
                                    rec["ex"][:, a2 - rel:b2 - rel],
                                    start=(k == 0), stop=(k == last_k[bi]))

                        rec.update(qk=qk, expaff=expaff, pv=pv)
                        if k == kmax - 1:
                            def norm(hst=hst, h=h):
                                ctx = hst["ctx"]
                                rc = nrm.tile([1, SH], F32, tag="rec")
                                nc.vector.reciprocal(rc[:], ctx[64:65, :])
                                rbc = nrm.tile([64, SH], F32, tag="rbc")
                                nc.gpsimd.partition_broadcast(rbc[:], rc[0:1, :])
                                po = 64 * (h % 2)
                                nc.vector.tensor_tensor(
                                    ctxT[po:po + 64, h // 2, s0:s0 + SH],
                                    ctx[0:64, :], rbc[:], ALU.mult)
                            rec["norm"] = norm
                        recs.append(rec)
                recs[0]["qk"]()
                for i, rec in enumerate(recs):
                    rec["expaff"]()
                    if i + 1 < len(recs):
                        recs[i + 1]["qk"]()
                    rec["pv"]()
                    if "norm" in rec:
                        rec["norm"]()
                    filler()

            # ---- attention (+ pieces 2-3 and first-half out-proj fillers) -
            with (
                tc.tile_pool(name="psc", bufs=2, space="PSUM") as psc,
                tc.tile_pool(name="pctx", bufs=1, space="PSUM") as pctx,
                tc.tile_pool(name="pwk", bufs=2, space="PSUM") as pwk,
            ):
                p2, p3 = NP // 2, NP // 2 + 1
                units_j0 = []
                st = {}
                for p in (p2, p3):
                    def mk_kv(p=p):
                        st[f"kvs{p}"] = unit_kv(pwk, p)
                    def mk_q8(p=p):
                        q8 = latp.tile([128, 2, 512], FP8, tag="q8")
                        st[f"q8{p}"] = q8
                        unit_q(pwk, p, 0, q8)
                    units_j0.append(mk_kv)
                    units_j0.append(mk_q8)
                    units_j0.append(lambda p=p: unit_q(pwk, p, 1, st[f"q8{p}"]))
                units_j0 += [(lambda j=j: unit_KT(pwk, p2, j, st[f"kvs{p2}"]))
                             for j in range(4)]
                units_j0 += [(lambda p=p, j=j: unit_QT(pwk, p, j, st[f"q8{p}"]))
                             for p in (p2, p3) for j in range(4)]
                units_j0 += [(lambda q=q: unit_V(pwk, p2, q, st[f"kvs{p2}"]))
                             for q in range(2)]
                f0 = Filler(units_j0, NHL * (SH // 128))
                attn_phase(0, range(NHL), f0, psc, pctx)
                f0.drain()

                # j1 fillers: burst of late-consumed evacs (first consumer is
                # chunk >= 8 of head 0), then first-half out-proj
                burst = [(lambda j=j: unit_KT(pwk, p3, j, st[f"kvs{p3}"]))
                         for j in range(4)]
                burst += [(lambda q=q: unit_V(pwk, p2, q, st[f"kvs{p2}"]))
                          for q in range(2, 4)]
                burst += [(lambda p=p, q=q: unit_V(pwk, p, q, st[f"kvs{p}"]))
                          for p in (p3,) for q in range(4)]
                units_E = [
                    (lambda si=si, o=o: unit_E(pwk, si, o))
                    for si in range(NT // 2) for o in range(2)
                ]
                fb = Filler(burst, 10)
                fe = Filler(units_E, NHL * (SH // 128) * 2 - 10)

                def f1():
                    if fb.units:
                        fb()
                    else:
                        fe()
                attn_phase(1, range(NHL), f1, psc, pctx)
                fb.drain()
                fe.drain()

            # ---- tail: remaining out-proj with a deep pool, ACT evacs -----
            with tc.tile_pool(name="ptl", bufs=4, space="PSUM") as ptl:
                for si in range(NT // 2, NT):
                    for o in range(2):
                        unit_E(ptl, si, o, on_act=True)

    nc.finalize()
    return nc


def _perm512():
    """Column permutation for w_kvu_k / w_qu so that PSUM chunk j, row
    32a+p corresponds to head 4*(j//2)+a, dim 32*(j%2)+p."""
    perm = np.empty(512, dtype=np.int64)
    for j in range(4):
        for a in range(4):
            for p in range(32):
                perm[128 * j + 32 * a + p] = 64 * (4 * (j // 2) + a) + 32 * (j % 2) + p
    return perm


def shard_inputs(inputs, S=2048):
    """Build the 8 per-core input maps from full inputs (host-side prep)."""
    f32 = lambda a: np.ascontiguousarray(np.asarray(a, dtype=np.float32))
    bf = lambda a: np.ascontiguousarray(
        np.asarray(a, dtype=np.float32).astype(ml_dtypes.bfloat16))
    fp8 = lambda a: np.ascontiguousarray(
        np.asarray(a, dtype=np.float32).astype(ml_dtypes.float8_e4m3))
    x = f32(inputs["x"])
    w_kvc, b_kvc = f32(inputs["w_kvc"]), f32(inputs["b_kvc"])
    w_kvu, b_kvu = f32(inputs["w_kvu"]), f32(inputs["b_kvu"])
    w_qc, b_qc = f32(inputs["w_qc"]), f32(inputs["b_qc"])
    w_qu, b_qu = f32(inputs["w_qu"]), f32(inputs["b_qu"])
    w_o, b_o = f32(inputs["w_o"]), f32(inputs["b_o"])
    perm = _perm512()
    in_maps = []
    for core in range(NCORES):
        b = core // 2
        g2 = core % 2
        ks = slice(512 * g2, 512 * g2 + 512)            # K-feature slice
        vs = slice(DIM + 512 * g2, DIM + 512 * g2 + 512)  # V-feature slice
        in_maps.append({
            "x": bf(x[b].T.reshape(ND, 128, S).transpose(1, 0, 2)),
            "x8": fp8(x[b].T.reshape(ND, 2, 64, S).transpose(2, 0, 1, 3)),
            "w_kvc": bf(w_kvc.reshape(ND, 128, LAT).transpose(1, 0, 2)),
            "w_qc8": fp8(w_qc.reshape(ND, 2, 64, QR).transpose(2, 0, 1, 3)),
            "w_kvu_k": bf(w_kvu[:, ks][:, perm]),
            "w_qu8": fp8(w_qu[:, ks][:, perm].reshape(2, 128, 512).transpose(1, 0, 2)),
            "w_kvu_v": bf(w_kvu[:, vs]),
            "w_o": bf(w_o[ks, :].reshape(4, 128, DIM).transpose(1, 0, 2)),
            "b_kvc": f32(b_kvc.reshape(LAT, 1)),
            "b_qc": f32(b_qc.reshape(2, 128).T),
            "b_qu": f32(b_qu[ks][perm].reshape(4, 128).T),
            "b_kvu_k": f32(b_kvu[ks][perm].reshape(4, 128).T),
            "b_kvu_v": f32(b_kvu[vs].reshape(1, 512)),
        })
    return in_maps


def kernel(**inputs) -> np.ndarray:
    from concourse.bass_utils import run_bass_kernel_spmd

    x = np.asarray(inputs["x"])
    S = x.shape[1]
    nc = build_mla(S=S)
    in_maps = shard_inputs(inputs, S=S)
    res = run_bass_kernel_spmd(nc, in_maps, list(range(NCORES))).results
    b_o = np.asarray(inputs["b_o"], dtype=np.float32)
    out = np.empty((B, S, DIM), dtype=np.float32)
    for b in range(B):
        out[b] = res[2 * b]["out"] + res[2 * b + 1]["out"] + b_o
    return out


# revision 15
# speedup vs baseline: 1.4442x; 1.0032x over previous
"""MLA (multi-head latent attention) Bass kernel for Trainium2, 8 NeuronCores.

Sharding: core i handles batch b = i // 2 and head-group g2 = i % 2
(8 of the 16 heads).  Each core computes a partial output
(its heads' contribution through out_proj); the host sums the two
partials per batch and adds b_o.

Design (ACT-bound; softmax exp on ScalarE is the per-core floor):
  - Host pre-lays-out everything: x transposed to bf16 xT [128,8,S] and
    fp8 x8T [64,8,2,S]; weights pre-cast (bf16 / fp8), K/Q up-projection
    columns pre-permuted so the fp8 DoubleRow layout falls out of plain
    PSUM evacuations.
  - QK^T runs in fp8e4 DoubleRow: KT8/QT8 stored [128p, g, plane, S]
    (partition 32a+p, plane pl = head 4g+a, dim 32pl+p); one matmul
    contracts all 64 head dims at 0.5 cycles/col.  The whole Q path
    (x->q_lat->QT) is fp8 DoubleRow too - it only feeds softmax scores,
    which tolerate fp8 noise.  V/out paths stay bf16.
  - Emission order software-pipelines: pieces 0-1 up front (deep scoped
    PSUM pool, KT/QT evacuations on the then-idle ACT engine), pieces
    2-3 as fillers inside j=0 attention, out-proj of the first token
    half as fillers inside j=1, remainder in a deep-pool tail with ACT
    evacuations.
  - PSUM: attention = scores [128,1024]x2bufs (4 banks) + ctx [65,1024]
    (2) + filler work tiles [128,512]x2 (2).
"""

import numpy as np
import ml_dtypes

import concourse.bass as bass
import concourse.bacc as bacc
import concourse.mybir as mybir
import concourse.tile as tile

DIM = 1024
NUM_HEADS = 16
HEAD_DIM = 64
LAT = 128
QR = 256
B = 4
NCORES = 8
ND = DIM // 128       # 8 d-chunks
NHL = 8               # heads per core
F32 = mybir.dt.float32
BF16 = mybir.dt.bfloat16
FP8 = mybir.dt.float8e4
AF = mybir.ActivationFunctionType
ALU = mybir.AluOpType
DR = mybir.MatmulPerfMode.DoubleRow


def _pieces(total, w=512):
    return [(o, min(w, total - o)) for o in range(0, total, w)]


def build_mla(S=2048):
    """Build the per-core Bass program (same SPMD program on all 8 cores)."""
    assert S % 1024 == 0
    SH = S // 2           # s-half width
    NT = S // 128         # number of 128-token chunks
    NP = S // 512         # number of 512-token pieces

    nc = bacc.Bacc()

    x_d = nc.declare_dram_parameter("x", [128, ND, S], BF16, isOutput=False)
    x8_d = nc.declare_dram_parameter("x8", [64, ND, 2, S], FP8, isOutput=False)
    w_kvc_d = nc.declare_dram_parameter("w_kvc", [128, ND, LAT], BF16, isOutput=False)
    w_qc8_d = nc.declare_dram_parameter("w_qc8", [64, ND, 2, QR], FP8, isOutput=False)
    w_kvu_k_d = nc.declare_dram_parameter("w_kvu_k", [128, 512], BF16, isOutput=False)
    w_qu8_d = nc.declare_dram_parameter("w_qu8", [128, 2, 512], FP8, isOutput=False)
    w_kvu_v_d = nc.declare_dram_parameter("w_kvu_v", [128, 512], BF16, isOutput=False)
    w_o_d = nc.declare_dram_parameter("w_o", [128, 4, DIM], BF16, isOutput=False)
    b_kvc_d = nc.declare_dram_parameter("b_kvc", [LAT, 1], F32, isOutput=False)
    b_qc_d = nc.declare_dram_parameter("b_qc", [128, 2], F32, isOutput=False)
    b_qu_d = nc.declare_dram_parameter("b_qu", [128, 4], F32, isOutput=False)
    b_kvu_k_d = nc.declare_dram_parameter("b_kvu_k", [128, 4], F32, isOutput=False)
    b_kvu_v_d = nc.declare_dram_parameter("b_kvu_v", [1, 512], F32, isOutput=False)
    out_d = nc.declare_dram_parameter("out", [S, DIM], F32, isOutput=True)

    with tile.TileContext(nc) as tc:
        with (
            tc.tile_pool(name="wts", bufs=1) as wts,
            tc.tile_pool(name="big", bufs=1) as big,
            tc.tile_pool(name="lat", bufs=2) as latp,
            tc.tile_pool(name="exb", bufs=4) as exb,
            tc.tile_pool(name="nrm", bufs=2) as nrm,
            tc.tile_pool(name="obp", bufs=4) as obp,
        ):
            # ---- early ACT-queue DMAs: weights for the first matmuls
            # lead, then biases (needed only at evac time); a dummy exp
            # preloads the activation table off the critical path --------
            w_kvc_sb = wts.tile([128, ND, LAT], BF16, name="w_kvc_sb")
            nc.scalar.dma_start(out=w_kvc_sb[:], in_=w_kvc_d[:, :, :])
            atl = wts.tile([1, 1], F32, name="atl")
            nc.gpsimd.memset(atl[:], 0.0)
            nc.scalar.activation(atl[:], atl[:], AF.Exp, scale=1.0)
            w_qc8_sb = wts.tile([64, ND, 2, QR], FP8, name="w_qc8_sb")
            nc.scalar.dma_start(out=w_qc8_sb[:], in_=w_qc8_d[:, :, :, :])
            w_qu8_sb = wts.tile([128, 2, 512], FP8, name="w_qu8_sb")
            nc.scalar.dma_start(out=w_qu8_sb[:], in_=w_qu8_d[:, :, :])
            w_kvu_k_sb = wts.tile([128, 512], BF16, name="w_kvu_k_sb")
            nc.scalar.dma_start(out=w_kvu_k_sb[:], in_=w_kvu_k_d[:, :])
            b_kvc_sb = wts.tile([128, 1], F32, name="b_kvc_sb")
            nc.scalar.dma_start(out=b_kvc_sb[:], in_=b_kvc_d[:, :])
            b_qc_sb = wts.tile([128, 2], F32, name="b_qc_sb")
            nc.scalar.dma_start(out=b_qc_sb[:], in_=b_qc_d[:, :])
            b_qu_sb = wts.tile([128, 4], F32, name="b_qu_sb")
            nc.scalar.dma_start(out=b_qu_sb[:], in_=b_qu_d[:, :])
            b_kvu_k_sb = wts.tile([128, 4], F32, name="b_kvu_k_sb")
            nc.scalar.dma_start(out=b_kvu_k_sb[:], in_=b_kvu_k_d[:, :])
            bv_row = wts.tile([1, 512], F32, name="bv_row")
            nc.scalar.dma_start(out=bv_row[:], in_=b_kvu_v_d[:, :])
            bvb = wts.tile([128, 512], F32, name="bvb")
            nc.gpsimd.partition_broadcast(bvb[:], bv_row[0:1, :])
            w_kvu_v_sb = wts.tile([128, 512], BF16, name="w_kvu_v_sb")
            nc.scalar.dma_start(out=w_kvu_v_sb[:], in_=w_kvu_v_d[:, :])

            # ---- xT / x8T on the SP queue, piece-major --------------------
            xT = big.tile([128, ND, S], BF16, name="xT")
            x8T = big.tile([64, ND, 2, S], FP8, name="x8T")
            for p in range(NP):
                nc.sync.dma_start(
                    out=xT[:, :, 512 * p:512 * p + 512],
                    in_=x_d[:, :, 512 * p:512 * p + 512])
                nc.sync.dma_start(
                    out=x8T[:, :, :, 512 * p:512 * p + 512],
                    in_=x8_d[:, :, :, 512 * p:512 * p + 512])

            # w_o rides the SP queue after xT/x8 (needed only in phase E)
            w_o_sb = wts.tile([128, 4, DIM], BF16, name="w_o_sb")
            nc.sync.dma_start(out=w_o_sb[:], in_=w_o_d[:, :, :])

            # ---- persistent tensors ---------------------------------------
            # KT8/QT8: [128p, g, plane, S]; partition 32a+p, plane pl
            # holds head 4g+a, dim 32*pl+p (fp8 for DoubleRow QK).
            KT8 = big.tile([128, 2, 2, S], FP8, name="KT8")
            QT8 = big.tile([128, 2, 2, S], FP8, name="QT8")
            # V: [128tok, chunk, head, 65] (64 vals + ones col)
            V = big.tile([128, NT, NHL, 65], BF16, name="V")
            nc.gpsimd.memset(V[:, :, :, 64:65], 1.0)
            # ctxT: [128 (2 heads x 64 dims), chunk h//2, S]
            ctxT = big.tile([128, 4, S], BF16, name="ctxT")

            # ---- work-unit emitters (pool + evac engine parameterized) ----
            def evac(on_act, dst, src, bias):
                if on_act:
                    nc.scalar.activation(dst, src, AF.Identity, bias=bias)
                else:
                    nc.vector.tensor_scalar_add(dst, src, bias)

            def unit_kv(pool, p):
                off = 512 * p
                kvp = pool.tile([128, 512], F32, tag="wk")
                for dc in range(ND):
                    nc.tensor.matmul(
                        kvp[:], w_kvc_sb[:, dc, :],
                        xT[:, dc, off:off + 512],
                        start=(dc == 0), stop=(dc == ND - 1))
                kvs = latp.tile([128, 512], BF16, tag="kvs")
                nc.vector.tensor_scalar_add(kvs[:], kvp[:], b_kvc_sb[:, 0:1])
                return kvs

            def unit_q(pool, p, qh, q8):
                off = 512 * p
                qp = pool.tile([128, 512], F32, tag="wk")
                for o in (0, 256):
                    for dc in range(ND):
                        nc.tensor.matmul(
                            qp[:, o:o + 256],
                            w_qc8_sb[:, dc, :, 128 * qh:128 * qh + 128],
                            x8T[:, dc, :, off + o:off + o + 256],
                            start=(dc == 0), stop=(dc == ND - 1),
                            perf_mode=DR)
                nc.vector.tensor_scalar_add(q8[:, qh, :], qp[:],
                                            b_qc_sb[:, qh:qh + 1])

            def unit_KT(pool, p, j, kvs, on_act=False):
                off = 512 * p
                kp = pool.tile([128, 512], F32, tag="wk")
                nc.tensor.matmul(kp[:], w_kvu_k_sb[:, 128 * j:128 * j + 128],
                                 kvs[:], start=True, stop=True)
                evac(on_act, KT8[:, j // 2, j % 2, off:off + 512], kp[:],
                     b_kvu_k_sb[:, j:j + 1])

            def unit_QT(pool, p, j, q8, on_act=False):
                off = 512 * p
                qp = pool.tile([128, 512], F32, tag="wk")
                for o in (0, 256):
                    nc.tensor.matmul(
                        qp[:, o:o + 256], w_qu8_sb[:, :, 128 * j:128 * j + 128],
                        q8[:, :, o:o + 256],
                        start=True, stop=True, perf_mode=DR)
                evac(on_act, QT8[:, j // 2, j % 2, off:off + 512], qp[:],
                     b_qu_sb[:, j:j + 1])

            def unit_V(pool, p, q, kvs):
                k = 4 * p + q
                vp = pool.tile([128, 512], F32, tag="wk")
                nc.tensor.matmul(vp[:], kvs[:, 128 * q:128 * q + 128],
                                 w_kvu_v_sb[:], start=True, stop=True)
                nc.vector.tensor_tensor(
                    V[:, k, :, 0:64],
                    vp[:].rearrange("p (h c) -> p h c", c=64),
                    bvb[:].rearrange("p (h c) -> p h c", c=64), ALU.add)

            def piece_units(pool, p, on_act=False, only=None):
                state = {}

                def mk_kv():
                    state["kvs"] = unit_kv(pool, p)

                def mk_q8():
                    q8 = latp.tile([128, 2, 512], FP8, tag="q8")
                    state["q8"] = q8
                    unit_q(pool, p, 0, q8)
                units = [("kv", mk_kv), ("q", mk_q8),
                         ("q", lambda: unit_q(pool, p, 1, state["q8"]))]
                units += [("KT", (lambda j=j: unit_KT(pool, p, j, state["kvs"],
                                                      on_act)))
                          for j in range(4)]
                units += [("QT", (lambda j=j: unit_QT(pool, p, j, state["q8"],
                                                      on_act)))
                          for j in range(4)]
                units += [("V", (lambda q=q: unit_V(pool, p, q, state["kvs"])))
                          for q in range(4)]
                for kind, u in units:
                    if only is None or kind in only:
                        yield u

            def unit_E(pool, si, o, dma_act=False):
                op = pool.tile([128, 512], F32, tag="wk")
                for cc in range(4):
                    nc.tensor.matmul(
                        op[:], ctxT[:, cc, 128 * si:128 * si + 128],
                        w_o_sb[:, cc, 512 * o:512 * o + 512],
                        start=(cc == 0), stop=(cc == 3))
                ob = obp.tile([128, 512], F32, tag="ob")
                nc.vector.tensor_copy(ob[:], op[:])
                eng = nc.scalar if dma_act else nc.sync
                eng.dma_start(
                    out=out_d[128 * si:128 * si + 128, 512 * o:512 * o + 512],
                    in_=ob[:])

            class Filler:
                """Dispenses queued work units evenly over `slots` calls."""
                def __init__(self, units, slots):
                    self.units = list(units)
                    self.slots = max(1, slots)
                    self.acc = 0.0
                    self.rate = len(self.units) / self.slots

                def __call__(self):
                    self.acc += self.rate
                    while self.acc >= 1.0 and self.units:
                        self.units.pop(0)()
                        self.acc -= 1.0

                def drain(self):
                    while self.units:
                        self.units.pop(0)()

            # ---- pieces 0..NP/2-1: deep scoped PSUM pool, ACT evacs -------
            with tc.tile_pool(name="pwk0", bufs=4, space="PSUM") as pwk0:
                for p in range(NP // 2):
                    for u in piece_units(pwk0, p, on_act=True):
                        u()


            def attn_phase(j, heads, filler, psc, pctx):
                """Attention for s-half j over `heads`, emitted with QK one
                chunk ahead of PV so exp never waits at head boundaries."""
                s0 = SH * j
                kmax = (SH // 128) * (j + 1)
                nbank = SH // 512
                last_k = {
                    bi: min(kmax - 1, (s0 + 512 * (bi + 1)) // 128 - 1)
                    for bi in range(nbank)
                }
                recs = []
                for h in heads:
                    g, a = h // 4, h % 4
                    hst = {}
                    for k in range(kmax):
                        t0 = 128 * k
                        ss = max(s0, t0)
                        fd = s0 + SH - ss
                        rel = ss - s0
                        rec = {}

                        def qk(rec=rec, g=g, a=a, t0=t0, ss=ss, fd=fd):
                            sc = psc.tile([128, SH], F32, tag="sc")
                            rec["sc"] = sc  # noqa
                            for o2, w2 in _pieces(fd, 256):
                                nc.tensor.matmul(
                                    sc[:, o2:o2 + w2],
                                    KT8[32 * a:32 * a + 32, g, :, t0:t0 + 128],
                                    QT8[32 * a:32 * a + 32, g, :,
                                        ss + o2:ss + o2 + w2],
                                    start=True, stop=True, perf_mode=DR,
                                    tile_position=(32 * a, 0))

                        def expaff(rec=rec, fd=fd, diag=(t0 >= s0)):
                            ex = exb.tile([128, SH], BF16, tag="ex")
                            rec["ex"] = ex  # noqa
                            nc.scalar.activation(ex[:, :fd], rec["sc"][:, :fd],
                                                 AF.Exp, scale=0.125)
                            if diag:
                                nc.gpsimd.affine_select(
                                    out=ex[:, 0:128], in_=ex[:, 0:128],
                                    pattern=[[1, 128]],
                                    compare_op=ALU.is_ge,
                                    fill=0.0, base=0, channel_multiplier=-1)

                        def pv(rec=rec, hst=hst, h=h, k=k, rel=rel):
                            if k == 0:
                                ctx = pctx.tile([65, SH], F32, tag="ctx")
                                hst["ctx"] = ctx
                            for bi in range(nbank):
                                a2 = max(rel, 512 * bi)
                                b2 = min(SH, 512 * bi + 512)
                                if a2 >= b2:
                                    continue
                                nc.tensor.matmul(
                                    hst["ctx"][:, a2:b2], V[:, k, h, :],
                                    rec["ex"][:, a2 - rel:b2 - rel],
                                    start=(k == 0), stop=(k == last_k[bi]))

                        rec.update(qk=qk, expaff=expaff, pv=pv)
                        if k == kmax - 1:
                            def norm(hst=hst, h=h):
                                ctx = hst["ctx"]
                                rc = nrm.tile([1, SH], F32, tag="rec")
                                nc.vector.reciprocal(rc[:], ctx[64:65, :])
                                rbc = nrm.tile([64, SH], F32, tag="rbc")
                                nc.gpsimd.partition_broadcast(rbc[:], rc[0:1, :])
                                po = 64 * (h % 2)
                                nc.vector.tensor_tensor(
                                    ctxT[po:po + 64, h // 2, s0:s0 + SH],
                                    ctx[0:64, :], rbc[:], ALU.mult)
                            rec["norm"] = norm
                        recs.append(rec)
                recs[0]["qk"]()
                for i, rec in enumerate(recs):
                    rec["expaff"]()
                    if i + 1 < len(recs):
                        recs[i + 1]["qk"]()
                    rec["pv"]()
                    if "norm" in rec:
                        rec["norm"]()
                    filler()

            # ---- attention (+ pieces 2-3 and first-half out-proj fillers) -
            with (
                tc.tile_pool(name="psc", bufs=2, space="PSUM") as psc,
                tc.tile_pool(name="pctx", bufs=1, space="PSUM") as pctx,
                tc.tile_pool(name="pwk", bufs=2, space="PSUM") as pwk,
            ):
                p2, p3 = NP // 2, NP // 2 + 1
                units_j0 = []
                st = {}
                for p in (p2, p3):
                    def mk_kv(p=p):
                        st[f"kvs{p}"] = unit_kv(pwk, p)
                    def mk_q8(p=p):
                        q8 = latp.tile([128, 2, 512], FP8, tag="q8")
                        st[f"q8{p}"] = q8
                        unit_q(pwk, p, 0, q8)
                    units_j0.append(mk_kv)
                    units_j0.append(mk_q8)
                    units_j0.append(lambda p=p: unit_q(pwk, p, 1, st[f"q8{p}"]))
                units_j0 += [(lambda p=p, j=j: unit_QT(pwk, p, j, st[f"q8{p}"]))
                             for p in (p2, p3) for j in range(4)]
                f0 = Filler(units_j0, 48)
                attn_phase(0, range(NHL), f0, psc, pctx)
                f0.drain()

                # j1 fillers: burst of late-consumed evacs (first consumer is
                # chunk >= 8 of head 0), then first-half out-proj
                burst = [(lambda p=p, j=j: unit_KT(pwk, p, j, st[f"kvs{p}"]))
                         for p in (p2, p3) for j in range(4)]
                burst += [(lambda p=p, q=q: unit_V(pwk, p, q, st[f"kvs{p}"]))
                          for p in (p2, p3) for q in range(4)]
                units_E = [
                    (lambda si=si, o=o: unit_E(pwk, si, o))
                    for si in range(NT // 2) for o in range(2)
                ]
                fb = Filler(burst, 7)
                fe = Filler(units_E, NHL * (SH // 128) * 2 - 7)

                def f1():
                    if fb.units:
                        fb()
                    else:
                        fe()
                attn_phase(1, range(NHL), f1, psc, pctx)
                fb.drain()
                fe.drain()

            # ---- tail: remaining out-proj with a deep pool, ACT evacs -----
            with tc.tile_pool(name="ptl", bufs=4, space="PSUM") as ptl:
                for i, si in enumerate(range(NT // 2, NT)):
                    for o in range(2):
                        unit_E(ptl, si, o, dma_act=(i % 4 == 3))

    nc.finalize()
    return nc


def _perm512():
    """Column permutation for w_kvu_k / w_qu so that PSUM chunk j, row
    32a+p corresponds to head 4*(j//2)+a, dim 32*(j%2)+p."""
    perm = np.empty(512, dtype=np.int64)
    for j in range(4):
        for a in range(4):
            for p in range(32):
                perm[128 * j + 32 * a + p] = 64 * (4 * (j // 2) + a) + 32 * (j % 2) + p
    return perm


def shard_inputs(inputs, S=2048):
    """Build the 8 per-core input maps from full inputs (host-side prep)."""
    f32 = lambda a: np.ascontiguousarray(np.asarray(a, dtype=np.float32))
    bf = lambda a: np.ascontiguousarray(
        np.asarray(a, dtype=np.float32).astype(ml_dtypes.bfloat16))
    fp8 = lambda a: np.ascontiguousarray(
        np.asarray(a, dtype=np.float32).astype(ml_dtypes.float8_e4m3))
    x = f32(inputs["x"])
    w_kvc, b_kvc = f32(inputs["w_kvc"]), f32(inputs["b_kvc"])
    w_kvu, b_kvu = f32(inputs["w_kvu"]), f32(inputs["b_kvu"])
    w_qc, b_qc = f32(inputs["w_qc"]), f32(inputs["b_qc"])
    w_qu, b_qu = f32(inputs["w_qu"]), f32(inputs["b_qu"])
    w_o, b_o = f32(inputs["w_o"]), f32(inputs["b_o"])
    perm = _perm512()
    in_maps = []
    for core in range(NCORES):
        b = core // 2
        g2 = core % 2
        ks = slice(512 * g2, 512 * g2 + 512)            # K-feature slice
        vs = slice(DIM + 512 * g2, DIM + 512 * g2 + 512)  # V-feature slice
        in_maps.append({
            "x": bf(x[b].T.reshape(ND, 128, S).transpose(1, 0, 2)),
            "x8": fp8(x[b].T.reshape(ND, 2, 64, S).transpose(2, 0, 1, 3)),
            "w_kvc": bf(w_kvc.reshape(ND, 128, LAT).transpose(1, 0, 2)),
            "w_qc8": fp8(w_qc.reshape(ND, 2, 64, QR).transpose(2, 0, 1, 3)),
            "w_kvu_k": bf(w_kvu[:, ks][:, perm]),
            "w_qu8": fp8(w_qu[:, ks][:, perm].reshape(2, 128, 512).transpose(1, 0, 2)),
            "w_kvu_v": bf(w_kvu[:, vs]),
            "w_o": bf(w_o[ks, :].reshape(4, 128, DIM).transpose(1, 0, 2)),
            "b_kvc": f32(b_kvc.reshape(LAT, 1)),
            "b_qc": f32(b_qc.reshape(2, 128).T),
            "b_qu": f32(b_qu[ks][perm].reshape(4, 128).T),
            "b_kvu_k": f32(b_kvu[ks][perm].reshape(4, 128).T),
            "b_kvu_v": f32(b_kvu[vs].reshape(1, 512)),
        })
    return in_maps


def kernel(**inputs) -> np.ndarray:
    from concourse.bass_utils import run_bass_kernel_spmd

    x = np.asarray(inputs["x"])
    S = x.shape[1]
    nc = build_mla(S=S)
    in_maps = shard_inputs(inputs, S=S)
    res = run_bass_kernel_spmd(nc, in_maps, list(range(NCORES))).results
    b_o = np.asarray(inputs["b_o"], dtype=np.float32)
    out = np.empty((B, S, DIM), dtype=np.float32)
    for b in range(B):
        out[b] = res[2 * b]["out"] + res[2 * b + 1]["out"] + b_o
    return out
